# revision 17
# baseline (speedup 1.0000x reference)
"""Trainium2 Bass kernel for nn_ContrastiveMROADMultiQueue.

Contract: kernel(**inputs) takes FULL unsharded inputs (as in
reference.setup_inputs()) and returns the full outputs
(q_cls, k_cls, new_queues, new_ptrs).

Sharding strategy (8 NeuronCores, SPMD single graph, per-core data):
  Phase A: row-parallel (batch*time/8 = 512 rows per core) layer1 + LayerNorm
           + ReLU for both encoders (key weights momentum-combined on device);
           AllGather of x^T; then column-parallel xi = x @ wih^T GEMM where
           each core computes exactly the 3H-slice (768 cols = r|z|n x 256)
           it needs for its share of the recurrence.
  Phase B: GRU recurrence with the hidden dim split 8 ways (256 rows of each
           gate per core). Query and key encoders are interleaved per step so
           each per-step AllGather of h-slices hides behind the other
           encoder's matmul. h is kept in transposed [H, B] layout end-to-end.
  Phase C: replicated head (relu -> wq -> l2norm) and a K-split (128 queue
           slots per core) scatter of k_cls into the per-class queues done
           with static-shape matmuls against host-built selection matrices.
"""

import numpy as np

NC_N = 8
B, T, DD, E, H = 32, 128, 2048, 1024, 2048
HS = H // NC_N            # 256 hidden rows per core
G3 = 3 * HS               # 768 gate rows per core
C, KQ, CD = 22, 1024, 128
KS = KQ // NC_N           # 128 queue slots per core
RP = B * T // NC_N        # 512 rows per core in phase A
BPC = B // NC_N           # 4 batches per core
MOM = 0.999
MASK_RATIO = 0.25
EPS = 1e-5

_CACHE = {}


def _build(t_steps=T, rp=RP):
    import concourse.bass as bass
    import concourse.mybir as mybir
    import concourse.tile as tile
    from concourse import bacc
    from concourse.masks import make_identity
    from contextlib import ExitStack

    fp32 = mybir.dt.float32
    f32r = mybir.dt.float32r
    AF = mybir.ActivationFunctionType
    OP = mybir.AluOpType
    RG = [list(range(NC_N))]

    nc = bacc.Bacc("TRN2", target_bir_lowering=False, debug=False,
                   num_devices=NC_N)

    def r(ap):
        return ap.bitcast(f32r)

    def bcast(ap, p=128):
        return bass.AP(tensor=ap.tensor, offset=ap.offset,
                       ap=[[0, p]] + list(ap.ap))

    rows_total = (rp * NC_N)
    n_rt = rp // 128           # row tiles per core (4)
    n_m = rows_total // 128    # row tiles across all cores (32)
    n_dc = 2 * DD // 128       # 32 contraction chunks for layer1
    n_ec = E // 128            # 8
    n_kc = H // 128            # 16

    # ---------------- inputs ----------------
    def inp(name, shape):
        return nc.dram_tensor(name, list(shape), fp32, kind="ExternalInput")

    rgb_rows = inp("rgb_rows", [rp, DD])
    flow_rows = inp("flow_rows", [rp, DD])
    mask_rows = inp("mask_rows", [rp])
    w1 = {"q": inp("w1_q", [2 * DD, E]), "k": inp("w1_k", [2 * DD, E])}
    b1 = {"q": inp("b1_q", [E]), "k": inp("b1_k", [E])}
    g_ = {"q": inp("g_q", [E]), "k": inp("g_k", [E])}
    be_ = {"q": inp("be_q", [E]), "k": inp("be_k", [E])}
    wih = {e: [inp(f"wih_{e}_{g}", [HS, E]) for g in range(3)]
           for e in ("q", "k")}
    whh = {e: [inp(f"whh_{e}_{g}", [HS, H]) for g in range(3)]
           for e in ("q", "k")}
    biasfold = {"q": inp("biasfold_q", [G3]), "k": inp("biasfold_k", [G3])}
    bhhn = {"q": inp("bhhn_q", [HS]), "k": inp("bhhn_k", [HS])}
    wq_in = inp("wq", [H, CD])
    bq_in = inp("bq", [CD])
    queues_in = inp("queues_j", [C, CD, KS])
    s_in = inp("S_j", [C, B, KS])
    mnot_in = inp("Mnot_j", [C, KS])

    q_cls_out = nc.dram_tensor("q_cls", [B, CD], fp32, kind="ExternalOutput")
    k_cls_out = nc.dram_tensor("k_cls", [B, CD], fp32, kind="ExternalOutput")
    newq_out = nc.dram_tensor("newq", [C, CD, KS], fp32,
                              kind="ExternalOutput")

    ENCS = ("q", "k")

    with tile.TileContext(nc) as tc, ExitStack() as ctx:
        const = ctx.enter_context(tc.tile_pool(name="const", bufs=1))
        dramp = ctx.enter_context(tc.tile_pool(name="dramp", bufs=1,
                                               space="DRAM"))
        agd = ctx.enter_context(tc.tile_pool(name="agd", bufs=4,
                                             space="DRAM"))
        bh = ctx.enter_context(tc.tile_pool(name="bh", bufs=2))

        # ---------------- persistent constants (small) ----------------
        ident = const.tile([128, 128], fp32, tag="ident", name="ident")
        make_identity(nc, ident[:, :])
        ones_src = const.tile([1, 128], fp32, tag="ones_src", name="ones_src")
        nc.vector.memset(ones_src[:, :], 1.0)
        ones_row = const.tile([1, 128], fp32, tag="ones_row", name="ones_row")
        nc.vector.tensor_copy(out=ones_row[:, :].bitcast(f32r),
                              in_=ones_src[:, :])
        bhhnb = {}
        for e in ENCS:
            bhhnb[e] = const.tile([32, HS], fp32, tag=f"bhhnb_{e}",
                                  name=f"bhhnb_{e}")
            nc.sync.dma_start(out=bhhnb[e][:, :],
                              in_=bcast(bhhn[e].ap(), p=32))
        bq_row = const.tile([1, CD], fp32, tag="bq_row", name="bq_row")
        nc.sync.dma_start(out=bq_row[:, :].bitcast(f32r),
                          in_=bq_in.ap().bitcast(f32r))
        wq_sb = const.tile([128, n_kc, CD], fp32, tag="wq_sb", name="wq_sb")
        nc.sync.dma_start(
            out=wq_sb[:, :, :].bitcast(f32r),
            in_=wq_in.ap().rearrange("(kc p) c -> p kc c",
                                     p=128).bitcast(f32r))

        # DRAM intermediates
        xcat_dram = dramp.tile([2 * DD, rp], fp32, tag="xcat_dram",
                               name="xcat_dram")
        xi_dram = {e: dramp.tile([rows_total, G3], fp32, tag=f"xi_{e}",
                                 name=f"xi_{e}") for e in ENCS}
        xall = {e: dramp.tile([NC_N, E, rp], fp32, tag=f"xall_{e}",
                              name=f"xall_{e}", addr_space="Shared")
                for e in ENCS}

        # ================= PHASE A =================
        with ExitStack() as actx:
            acst = actx.enter_context(tc.tile_pool(name="acst", bufs=1))
            tmp = actx.enter_context(tc.tile_pool(name="tmpA", bufs=2))
            lnp = actx.enter_context(tc.tile_pool(name="lnp", bufs=4))
            a2 = actx.enter_context(tc.tile_pool(name="a2", bufs=2))
            xl = actx.enter_context(tc.tile_pool(name="xl", bufs=4))
            wprep = actx.enter_context(tc.tile_pool(name="wprep", bufs=1))
            xsb = actx.enter_context(tc.tile_pool(name="xsb", bufs=1))

            # phase-A constants
            mask_b = acst.tile([128, rp], fp32, tag="mask_b", name="mask_b")
            nc.sync.dma_start(out=mask_b[:, :], in_=bcast(mask_rows.ap()))
            epst = acst.tile([128, 1], fp32, tag="epst", name="epst")
            nc.vector.memset(epst[:, :], EPS)
            gb, beb, b1row, bfrow = {}, {}, {}, {}
            for e in ENCS:
                gb[e] = acst.tile([128, E], fp32, tag=f"gb_{e}",
                                  name=f"gb_{e}")
                nc.sync.dma_start(out=gb[e][:, :], in_=bcast(g_[e].ap()))
                beb[e] = acst.tile([128, E], fp32, tag=f"beb_{e}",
                                   name=f"beb_{e}")
                nc.sync.dma_start(out=beb[e][:, :], in_=bcast(be_[e].ap()))
                b1row[e] = acst.tile([1, E], fp32, tag=f"b1row_{e}",
                                     name=f"b1row_{e}")
                nc.sync.dma_start(out=b1row[e][:, :].bitcast(f32r),
                                  in_=b1[e].ap().bitcast(f32r))
                bfrow[e] = acst.tile([1, G3], fp32, tag=f"bfrow_{e}",
                                     name=f"bfrow_{e}")
                nc.sync.dma_start(out=bfrow[e][:, :].bitcast(f32r),
                                  in_=biasfold[e].ap().bitcast(f32r))

            # ---- A1: transpose inputs into xcat_dram [2D, rp] ----
            with tc.tile_pool(name="tpsA1", bufs=4, space="PSUM") as tps1:
                for kc in range(n_dc):
                    src = rgb_rows if kc < DD // 128 else flow_rows
                    dcol = (kc % (DD // 128)) * 128
                    slab = tmp.tile([128, n_rt, 128], fp32, tag="slab",
                                    name="slab")
                    nc.sync.dma_start(
                        out=slab[:, :, :],
                        in_=src[:, dcol:dcol + 128]
                        .rearrange("(rt p) d -> p rt d", p=128))
                    xc_sb = tmp.tile([128, rp], fp32, tag="xc_sb",
                                     name="xc_sb")
                    for rt in range(n_rt):
                        ps = tps1.tile([128, 128], fp32, tag="tpsa",
                                       name="tpsa")
                        nc.tensor.transpose(ps[:, :], slab[:, rt, :],
                                            ident[:, :])
                        nc.vector.tensor_copy(
                            out=xc_sb[:, rt * 128:(rt + 1) * 128],
                            in_=ps[:, :])
                    nc.sync.dma_start(
                        out=xcat_dram[kc * 128:(kc + 1) * 128, :],
                        in_=xc_sb[:, :])

            # ---- layer1 + LN (x stays in SBUF), per encoder ----
            x_sb = {e: xsb.tile([128, n_rt, E], fp32, tag=f"x_sb_{e}",
                                name=f"x_sb_{e}") for e in ENCS}
            l1scope = ExitStack()
            l1ps = l1scope.enter_context(
                tc.tile_pool(name="l1ps", bufs=1, space="PSUM"))
            for e in ENCS:
                psl = [[l1ps.tile([128, 512], fp32, tag=f"l1_{rt}_{n}",
                                  name=f"l1_{rt}_{n}")
                        for n in range(2)] for rt in range(n_rt)]
                for kc in range(n_dc):
                    xck = tmp.tile([128, rp], fp32, tag="xck", name="xck")
                    nc.sync.dma_start(
                        out=xck[:, :].bitcast(f32r),
                        in_=xcat_dram[kc * 128:(kc + 1) * 128,
                                      :].bitcast(f32r))
                    if e == "q":
                        xmm = tmp.tile([128, rp], fp32, tag="xmm",
                                       name="xmm")
                        nc.vector.tensor_mul(xmm[:, :].bitcast(f32r),
                                             xck[:, :], mask_b[:, :])
                        src_chunk = xmm
                    else:
                        src_chunk = xck
                    w1row = tmp.tile([128, E], fp32, tag="w1row",
                                     name="w1row")
                    nc.sync.dma_start(
                        out=w1row[:, :].bitcast(f32r),
                        in_=w1["q"][kc * 128:(kc + 1) * 128,
                                    :].bitcast(f32r))
                    if e == "k":
                        w1k = tmp.tile([128, E], fp32, tag="w1k", name="w1k")
                        nc.sync.dma_start(
                            out=w1k[:, :],
                            in_=w1["k"][kc * 128:(kc + 1) * 128, :])
                        w1c = tmp.tile([128, E], fp32, tag="w1c", name="w1c")
                        nc.vector.tensor_scalar_mul(w1c[:, :].bitcast(f32r),
                                                    w1k[:, :], MOM)
                        nc.vector.scalar_tensor_tensor(
                            out=w1c[:, :].bitcast(f32r), in0=w1row[:, :],
                            scalar=1.0 - MOM,
                            in1=w1c[:, :], op0=OP.mult, op1=OP.add)
                        wrow = w1c
                    else:
                        wrow = w1row
                    for rt in range(n_rt):
                        for n in range(2):
                            nc.tensor.matmul(
                                psl[rt][n][:, :],
                                r(src_chunk[:, rt * 128:(rt + 1) * 128]),
                                r(wrow[:, n * 512:(n + 1) * 512]),
                                start=(kc == 0), stop=False)
                for rt in range(n_rt):
                    for n in range(2):
                        nc.tensor.matmul(
                            psl[rt][n][:, :], r(ones_row[:, 0:128]),
                            r(b1row[e][:, n * 512:(n + 1) * 512]),
                            start=False, stop=True)
                # LN + affine into x_sb (relu folded into transpose copy)
                for rt in range(n_rt):
                    stats = lnp.tile([128, 2, 6], fp32, tag="stats",
                                     name="stats")
                    nc.vector.bn_stats(out=stats[:, 0, :],
                                       in_=psl[rt][0][:, :])
                    nc.vector.bn_stats(out=stats[:, 1, :],
                                       in_=psl[rt][1][:, :])
                    mv = lnp.tile([128, 2], fp32, tag="mv", name="mv")
                    nc.vector.bn_aggr(out=mv[:, :], in_=stats[:, :, :])
                    rstd = lnp.tile([128, 1], fp32, tag="rstd", name="rstd")
                    nc.scalar.activation(out=rstd[:, :], in_=mv[:, 1:2],
                                         func=AF.Sqrt, bias=epst[:, :],
                                         scale=1.0)
                    nc.vector.reciprocal(out=rstd[:, :], in_=rstd[:, :])
                    nmr = lnp.tile([128, 1], fp32, tag="nmr", name="nmr")
                    nc.vector.scalar_tensor_tensor(
                        out=nmr[:, :], in0=mv[:, 0:1], scalar=-1.0,
                        in1=rstd[:, :], op0=OP.mult, op1=OP.mult)
                    t1 = a2.tile([128, E], fp32, tag="t1", name="t1")
                    for n in range(2):
                        nc.scalar.activation(
                            out=t1[:, n * 512:(n + 1) * 512],
                            in_=psl[rt][n][:, :], func=AF.Identity,
                            bias=nmr[:, :], scale=rstd[:, :])
                    nc.vector.tensor_mul(t1[:, :], t1[:, :], gb[e][:, :])
                    nc.vector.tensor_add(x_sb[e][:, rt, :], t1[:, :],
                                         beb[e][:, :])
            l1scope.close()

            # ---- x^T transposes (relu folded) + AllGather, per encoder ----
            tps2 = actx.enter_context(tc.tile_pool(name="tps2", bufs=4,
                                                   space="PSUM"))
            for e in ENCS:
                agi = agd.tile([E, rp], fp32, tag="agx_in", name="agx_in")
                for rt in range(n_rt):
                    xtp = a2.tile([128, n_ec, 128], fp32, tag="xtp",
                                  name="xtp")
                    for ec in range(n_ec):
                        ps = tps2.tile([128, 128], fp32, tag="tpsa2",
                                       name="tpsa2")
                        nc.tensor.transpose(
                            ps[:, :],
                            x_sb[e][:, rt, ec * 128:(ec + 1) * 128],
                            ident[:, :])
                        nc.vector.tensor_scalar_max(xtp[:, ec, :],
                                                    ps[:, :], 0.0)
                    nc.sync.dma_start(
                        out=agi[:, rt * 128:(rt + 1) * 128]
                        .rearrange("(ec p) c -> p ec c", p=128),
                        in_=xtp[:, :, :])
                nc.gpsimd.collective_compute(
                    "AllGather", OP.bypass, replica_groups=RG,
                    ins=[agi[:, :].opt()], outs=[xall[e][:, :, :].opt()])

            # ---- wih^T prep + xi GEMM, per encoder ----
            xip = actx.enter_context(tc.tile_pool(name="xip", bufs=2,
                                                  space="PSUM"))
            for e in ENCS:
                wihT = wprep.tile([128, n_ec, G3], fp32, tag="wihT",
                                  name="wihT")
                for g in range(3):
                    for rt2 in range(HS // 128):
                        wt = tmp.tile([128, E], fp32, tag="w1row",
                                      name="wihrow")
                        nc.sync.dma_start(
                            out=wt[:, :],
                            in_=wih["q"][g][rt2 * 128:(rt2 + 1) * 128, :])
                        if e == "k":
                            wtk = tmp.tile([128, E], fp32, tag="w1k",
                                           name="wihrowk")
                            nc.sync.dma_start(
                                out=wtk[:, :],
                                in_=wih["k"][g][rt2 * 128:(rt2 + 1) * 128, :])
                            wtc = tmp.tile([128, E], fp32, tag="w1c",
                                           name="wihrowc")
                            nc.vector.tensor_scalar_mul(wtc[:, :], wtk[:, :],
                                                        MOM)
                            nc.vector.scalar_tensor_tensor(
                                out=wtc[:, :], in0=wt[:, :],
                                scalar=1.0 - MOM, in1=wtc[:, :],
                                op0=OP.mult, op1=OP.add)
                            wt = wtc
                        for ec in range(n_ec):
                            ps = tps2.tile([128, 128], fp32, tag="tpsa2",
                                           name="tpsa2")
                            nc.tensor.transpose(
                                ps[:, :], wt[:, ec * 128:(ec + 1) * 128],
                                ident[:, :])
                            nc.vector.tensor_copy(
                                out=wihT[:, ec,
                                         g * HS + rt2 * 128:
                                         g * HS + (rt2 + 1) * 128]
                                .bitcast(f32r),
                                in_=ps[:, :])
                for m in range(n_m):
                    jb, sub = divmod(m, n_rt)
                    psA = xip.tile([128, 512], fp32, tag="xipA", name="xipA")
                    psB = xip.tile([128, 256], fp32, tag="xipB", name="xipB")
                    for kc in range(n_ec):
                        lt = xl.tile([128, 128], fp32, tag="lt", name="lt")
                        nc.sync.dma_start(
                            out=lt[:, :].bitcast(f32r),
                            in_=xall[e][jb, kc * 128:(kc + 1) * 128,
                                        sub * 128:(sub + 1) * 128]
                            .bitcast(f32r))
                        nc.tensor.matmul(psA[:, :], r(lt[:, :]),
                                         r(wihT[:, kc, 0:512]),
                                         start=(kc == 0), stop=False)
                        nc.tensor.matmul(psB[:, :], r(lt[:, :]),
                                         r(wihT[:, kc, 512:G3]),
                                         start=(kc == 0), stop=False)
                    nc.tensor.matmul(psA[:, :], r(ones_row[:, 0:128]),
                                     r(bfrow[e][:, 0:512]),
                                     start=False, stop=True)
                    nc.tensor.matmul(psB[:, :], r(ones_row[:, 0:128]),
                                     r(bfrow[e][:, 512:G3]),
                                     start=False, stop=True)
                    xi_sb = a2.tile([128, G3], fp32, tag="xi_sb",
                                    name="xi_sb")
                    nc.vector.tensor_copy(out=xi_sb[:, 0:512], in_=psA[:, :])
                    nc.vector.tensor_copy(out=xi_sb[:, 512:G3],
                                          in_=psB[:, :])
                    nc.sync.dma_start(
                        out=xi_dram[e][m * 128:(m + 1) * 128, :],
                        in_=xi_sb[:, :])

        # ================= whh^T prep (after phase A frees SBUF) =========
        wscope = ctx.enter_context(tc.tile_pool(name="wscope", bufs=1))
        whhT = {e: wscope.tile([128, n_kc, G3], fp32, tag=f"whhT_{e}",
                               name=f"whhT_{e}") for e in ENCS}
        with ExitStack() as pctx:
            ptmp = pctx.enter_context(tc.tile_pool(name="ptmp", bufs=2))
            ptps = pctx.enter_context(tc.tile_pool(name="ptps", bufs=4,
                                                   space="PSUM"))
            for e in ENCS:
                for g in range(3):
                    for rt2 in range(HS // 128):
                        wt = ptmp.tile([128, H], fp32, tag="whhrow",
                                       name="whhrow")
                        nc.sync.dma_start(
                            out=wt[:, :],
                            in_=whh["q"][g][rt2 * 128:(rt2 + 1) * 128, :])
                        if e == "k":
                            wtk = ptmp.tile([128, H], fp32, tag="whhrowk",
                                            name="whhrowk")
                            nc.sync.dma_start(
                                out=wtk[:, :],
                                in_=whh["k"][g][rt2 * 128:(rt2 + 1) * 128, :])
                            wtc = ptmp.tile([128, H], fp32, tag="whhrowc",
                                            name="whhrowc")
                            nc.vector.tensor_scalar_mul(wtc[:, :], wtk[:, :],
                                                        MOM)
                            nc.vector.scalar_tensor_tensor(
                                out=wtc[:, :], in0=wt[:, :],
                                scalar=1.0 - MOM, in1=wtc[:, :],
                                op0=OP.mult, op1=OP.add)
                            wt = wtc
                        for kc in range(n_kc):
                            ps = ptps.tile([128, 128], fp32, tag="ptpsa",
                                           name="ptpsa")
                            nc.tensor.transpose(
                                ps[:, :], wt[:, kc * 128:(kc + 1) * 128],
                                ident[:, :])
                            nc.vector.tensor_copy(
                                out=whhT[e][:, kc,
                                            g * HS + rt2 * 128:
                                            g * HS + (rt2 + 1) * 128]
                                .bitcast(f32r),
                                in_=ps[:, :])

        # ================= PHASE B: recurrence =================
        with ExitStack() as bctx:
            bxi = bctx.enter_context(tc.tile_pool(name="bxi", bufs=2))
            bgate = bctx.enter_context(tc.tile_pool(name="bgate", bufs=2))
            bps = bctx.enter_context(tc.tile_pool(name="bps", bufs=1,
                                                  space="PSUM"))
            btps = bctx.enter_context(tc.tile_pool(name="btps", bufs=2,
                                                   space="PSUM"))
            hT = {e: None for e in ENCS}
            h_prev = {e: None for e in ENCS}
            xi_r = {e: xi_dram[e][:, :].rearrange("(b t) g -> t b g",
                                                  t=t_steps)
                    for e in ENCS}
            for t in range(t_steps):
                for e in ENCS:
                    xi_t = bxi.tile([32, G3], fp32, tag=f"xi_t_{e}",
                                    name=f"xi_t_{e}")
                    nc.sync.dma_start(out=xi_t[:, :], in_=xi_r[e][t])
                    trz = bgate.tile([32, 512], fp32, tag=f"trz_{e}",
                                     name=f"trz_{e}")
                    tn = bgate.tile([32, HS], fp32, tag=f"tn_{e}",
                                    name=f"tn_{e}")
                    if t > 0:
                        ps_rz = bps.tile([32, 512], fp32, tag=f"psrz_{e}",
                                         name=f"psrz_{e}")
                        ps_n = bps.tile([32, 256], fp32, tag=f"psn_{e}",
                                        name=f"psn_{e}")
                        for kc in range(n_kc):
                            nc.tensor.matmul(
                                ps_rz[:, :], r(hT[e][:, kc, :]),
                                r(whhT[e][:, kc, 0:512]),
                                start=(kc == 0), stop=(kc == n_kc - 1))
                            nc.tensor.matmul(
                                ps_n[:, :], r(hT[e][:, kc, :]),
                                r(whhT[e][:, kc, 512:G3]),
                                start=(kc == 0), stop=(kc == n_kc - 1))
                        nc.vector.tensor_add(trz[:, :], ps_rz[:, :],
                                             xi_t[:, 0:512])
                        nc.vector.tensor_add(tn[:, :], ps_n[:, :],
                                             bhhnb[e][:, :])
                    else:
                        nc.vector.tensor_copy(out=trz[:, :],
                                              in_=xi_t[:, 0:512])
                        nc.vector.tensor_copy(out=tn[:, :],
                                              in_=bhhnb[e][:, :])
                    rz = bgate.tile([32, 512], fp32, tag=f"rz_{e}",
                                    name=f"rz_{e}")
                    nc.scalar.activation(out=rz[:, :], in_=trz[:, :],
                                         func=AF.Sigmoid)
                    nc.vector.tensor_mul(tn[:, :], rz[:, 0:256], tn[:, :])
                    nc.vector.tensor_add(tn[:, :], tn[:, :], xi_t[:, 512:G3])
                    nn_t = bgate.tile([32, HS], fp32, tag=f"nn_{e}",
                                      name=f"nn_{e}")
                    nc.scalar.activation(out=nn_t[:, :], in_=tn[:, :],
                                         func=AF.Tanh)
                    d_t = bgate.tile([32, HS], fp32, tag=f"d_{e}",
                                     name=f"d_{e}")
                    if t > 0:
                        nc.vector.tensor_sub(d_t[:, :], h_prev[e][:, :],
                                             nn_t[:, :])
                    else:
                        nc.vector.tensor_scalar_mul(d_t[:, :], nn_t[:, :],
                                                    -1.0)
                    nc.vector.tensor_mul(d_t[:, :], rz[:, 256:512],
                                         d_t[:, :])
                    h_new = bh.tile([32, HS], fp32, tag=f"hnew_{e}",
                                    name=f"hnew_{e}")
                    nc.vector.tensor_add(h_new[:, :], nn_t[:, :], d_t[:, :])
                    h_prev[e] = h_new
                    # transpose h slice + AllGather
                    agi = agd.tile([HS, 32], fp32, tag=f"agh_in_{e}",
                                   name=f"agh_in_{e}")
                    hts = bgate.tile([128, HS // 128, 32], fp32,
                                     tag=f"hts_{e}", name=f"hts_{e}")
                    for hf in range(HS // 128):
                        pst = btps.tile([128, 32], fp32, tag="pst",
                                        name="pst")
                        nc.tensor.transpose(
                            pst[:, :], h_new[:, hf * 128:(hf + 1) * 128],
                            ident[0:32, 0:32])
                        nc.vector.tensor_copy(out=hts[:, hf, :],
                                              in_=pst[:, :])
                    nc.sync.dma_start(
                        out=agi[:, :].rearrange("(hf p) c -> p hf c", p=128),
                        in_=hts[:, :, :])
                    ago = agd.tile([H, 32], fp32, tag=f"agh_out_{e}",
                                   name=f"agh_out_{e}", addr_space="Shared")
                    nc.gpsimd.collective_compute(
                        "AllGather", OP.bypass, replica_groups=RG,
                        ins=[agi[:, :].opt()], outs=[ago[:, :].opt()])
                    hT_new = bh.tile([128, n_kc, 32], fp32, tag=f"hT_{e}",
                                     name=f"hT_{e}")
                    nc.sync.dma_start(
                        out=hT_new[:, :, :].bitcast(f32r),
                        in_=ago[:, :].rearrange("(kc p) c -> p kc c",
                                                p=128).bitcast(f32r))
                    hT[e] = hT_new

        # ================= PHASE C: head + queues =================
        with ExitStack() as cctx:
            cp = cctx.enter_context(tc.tile_pool(name="cp", bufs=2))
            cps = cctx.enter_context(tc.tile_pool(name="cps", bufs=2,
                                                  space="PSUM"))
            qt = cctx.enter_context(tc.tile_pool(name="qt", bufs=3))
            qconst = cctx.enter_context(tc.tile_pool(name="qconst", bufs=1))

            s_sb = qconst.tile([B, C, KS], fp32, tag="s_sb", name="s_sb")
            nc.sync.dma_start(
                out=s_sb[:, :, :].bitcast(f32r),
                in_=s_in.ap().rearrange("c b k -> b c k").bitcast(f32r))
            mnot_b = qconst.tile([128, C, KS], fp32, tag="mnot_b",
                                 name="mnot_b")
            nc.sync.dma_start(out=mnot_b[:, :, :], in_=bcast(mnot_in.ap()))
            qsb = qconst.tile([128, C, KS], fp32, tag="qsb", name="qsb")
            nc.sync.dma_start(out=qsb[:, :, :],
                              in_=queues_in.ap().rearrange("c p k -> p c k"))

            cls_sb = {}
            for e in ENCS:
                featT = cp.tile([128, n_kc, 32], fp32, tag=f"featT_{e}",
                                name=f"featT_{e}")
                nc.scalar.activation(out=featT[:, :, :].bitcast(f32r),
                                     in_=hT[e][:, :, :], func=AF.Relu)
                ps_cls = cps.tile([32, CD], fp32, tag=f"pscls_{e}",
                                  name=f"pscls_{e}")
                for kc in range(n_kc):
                    nc.tensor.matmul(ps_cls[:, :], r(featT[:, kc, :]),
                                     r(wq_sb[:, kc, :]),
                                     start=(kc == 0), stop=False)
                nc.tensor.matmul(ps_cls[:, :], r(ones_row[:, 0:32]),
                                 r(bq_row[:, :]), start=False, stop=True)
                sq = cp.tile([32, CD], fp32, tag=f"sq_{e}", name=f"sq_{e}")
                ssum = cp.tile([32, 1], fp32, tag=f"ssum_{e}",
                               name=f"ssum_{e}")
                nc.scalar.activation(out=sq[:, :], in_=ps_cls[:, :],
                                     func=AF.Square, accum_out=ssum[:, :])
                rn = cp.tile([32, 1], fp32, tag=f"rn_{e}", name=f"rn_{e}")
                nc.scalar.activation(out=rn[:, :], in_=ssum[:, :],
                                     func=AF.Sqrt)
                nc.vector.reciprocal(out=rn[:, :], in_=rn[:, :])
                cls = cp.tile([32, CD], fp32, tag=f"cls_{e}",
                              name=f"cls_{e}")
                nc.scalar.activation(out=cls[:, :], in_=ps_cls[:, :],
                                     func=AF.Copy, scale=rn[:, :])
                cls_sb[e] = cls
                out_t = q_cls_out if e == "q" else k_cls_out
                nc.sync.dma_start(out=out_t[:, :], in_=cls[:, :])

            clsr = cp.tile([32, CD], fp32, tag="clsr", name="clsr")
            nc.vector.tensor_copy(out=clsr[:, :].bitcast(f32r),
                                  in_=cls_sb["k"][:, :])
            for c in range(C):
                psq = cps.tile([128, KS], fp32, tag="psq", name="psq")
                nc.tensor.matmul(psq[:, :], r(clsr[:, :]),
                                 r(s_sb[:, c, :]), start=True, stop=True)
                qn = qt.tile([128, KS], fp32, tag="qn", name="qn")
                nc.vector.tensor_mul(qn[:, :], qsb[:, c, :],
                                     mnot_b[:, c, :])
                nc.vector.tensor_add(qn[:, :], qn[:, :], psq[:, :])
                nc.sync.dma_start(out=newq_out[c, :, :], in_=qn[:, :])

    nc.compile()
    return nc


def _prep_inputs(inputs, t_steps=T, rp=RP):
    """Build the 8 per-core input maps from the full input dict."""
    f = {k: np.asarray(v) for k, v in inputs.items()}
    rgb = f["rgb"].astype(np.float32, copy=False)
    flow = f["flow"].astype(np.float32, copy=False)
    rand_mask = f["rand_mask"].astype(np.float32, copy=False)
    targets = f["targets"].astype(np.float32, copy=False)
    ptrs = f["ptrs"].astype(np.int64)
    queues = f["queues"].astype(np.float32, copy=False)

    b_ = rgb.shape[0]
    mask = (rand_mask[:, :, 0] > MASK_RATIO).astype(np.float32)
    mask[:, -1] = 1.0

    def mu(kp, qp):
        return (MOM * f[kp].astype(np.float64)
                + (1.0 - MOM) * f[qp].astype(np.float64)).astype(np.float32)

    b1_k = mu("b1_k", "b1_q")
    g_k = mu("g_k", "g_q")
    be_k = mu("be_k", "be_q")
    bih_k = mu("bih_k", "bih_q")
    bhh_k = mu("bhh_k", "bhh_q")
    bih_q = f["bih_q"].astype(np.float32, copy=False)
    bhh_q = f["bhh_q"].astype(np.float32, copy=False)

    sel = targets > 0.5
    pos = np.cumsum(sel, axis=0) - 1
    slot = (ptrs[None, :].astype(np.int64) + pos) % KQ
    cnt = sel.sum(0).astype(np.int64)
    new_ptrs = ((ptrs + cnt) % KQ).astype(np.int32)

    S = np.zeros((NC_N, C, b_, KS), np.float32)
    for bb in range(b_):
        for cc in range(C):
            if sel[bb, cc]:
                s = int(slot[bb, cc])
                S[s // KS, cc, bb, s % KS] = 1.0
    Mnot = 1.0 - S.sum(axis=2)  # [NC, C, KS]

    bpc = b_ // NC_N
    in_maps = []
    for j in range(NC_N):
        m = {
            "rgb_rows": np.ascontiguousarray(
                rgb[j * bpc:(j + 1) * bpc].reshape(rp, -1)),
            "flow_rows": np.ascontiguousarray(
                flow[j * bpc:(j + 1) * bpc].reshape(rp, -1)),
            "mask_rows": np.ascontiguousarray(
                mask[j * bpc:(j + 1) * bpc].reshape(rp)),
            "w1_q": f["w1_q"], "w1_k": f["w1_k"],
            "b1_q": f["b1_q"], "b1_k": b1_k,
            "g_q": f["g_q"], "g_k": g_k,
            "be_q": f["be_q"], "be_k": be_k,
            "wq": f["wq"], "bq": f["bq"],
            "queues_j": np.ascontiguousarray(
                queues[:, :, j * KS:(j + 1) * KS]),
            "S_j": np.ascontiguousarray(S[j]),
            "Mnot_j": np.ascontiguousarray(Mnot[j]),
        }
        hh = H
        for e in ("q", "k"):
            wihf = f[f"wih_{e}"]
            whhf = f[f"whh_{e}"]
            for g in range(3):
                sl = slice(g * hh + j * HS, g * hh + (j + 1) * HS)
                m[f"wih_{e}_{g}"] = np.ascontiguousarray(wihf[sl])
                m[f"whh_{e}_{g}"] = np.ascontiguousarray(whhf[sl])
        for e, bihv, bhhv in (("q", bih_q, bhh_q), ("k", bih_k, bhh_k)):
            bf = np.empty(G3, np.float32)
            for g in range(3):
                sl = slice(g * hh + j * HS, g * hh + (j + 1) * HS)
                bf[g * HS:(g + 1) * HS] = bihv[sl]
                if g < 2:  # bhh for r,z folded; n-gate bhh applied in-step
                    bf[g * HS:(g + 1) * HS] += bhhv[sl]
            m[f"biasfold_{e}"] = bf
            m[f"bhhn_{e}"] = np.ascontiguousarray(
                bhhv[2 * hh + j * HS: 2 * hh + (j + 1) * HS])
        in_maps.append(m)
    return in_maps, new_ptrs


def _assemble(results, new_ptrs):
    q_cls = np.asarray(results[0]["q_cls"])
    k_cls = np.asarray(results[0]["k_cls"])
    new_queues = np.concatenate(
        [np.asarray(results[j]["newq"]) for j in range(NC_N)], axis=2)
    return q_cls, k_cls, new_queues, new_ptrs


def kernel(**inputs):
    from concourse import bass_utils
    if "nc" not in _CACHE:
        _CACHE["nc"] = _build()
    nc = _CACHE["nc"]
    in_maps, new_ptrs = _prep_inputs(inputs)
    res = bass_utils.run_bass_kernel_spmd(nc, in_maps,
                                          core_ids=list(range(NC_N)))
    return _assemble(res.results, new_ptrs)


# revision 19
# speedup vs baseline: 1.0280x; 1.0280x over previous
"""Trainium2 Bass kernel for nn_ContrastiveMROADMultiQueue.

Contract: kernel(**inputs) takes FULL unsharded inputs (as in
reference.setup_inputs()) and returns the full outputs
(q_cls, k_cls, new_queues, new_ptrs).

Sharding strategy (8 NeuronCores, SPMD single graph, per-core data):
  Phase A: row-parallel (batch*time/8 = 512 rows per core) layer1 + LayerNorm
           + ReLU for both encoders (key weights momentum-combined on device);
           AllGather of x^T; then column-parallel xi = x @ wih^T GEMM where
           each core computes exactly the 3H-slice (768 cols = r|z|n x 256)
           it needs for its share of the recurrence.
  Phase B: GRU recurrence with the hidden dim split 8 ways (256 rows of each
           gate per core). Query and key encoders are interleaved per step so
           each per-step AllGather of h-slices hides behind the other
           encoder's matmul. h is kept in transposed [H, B] layout end-to-end.
  Phase C: replicated head (relu -> wq -> l2norm) and a K-split (128 queue
           slots per core) scatter of k_cls into the per-class queues done
           with static-shape matmuls against host-built selection matrices.
"""

import numpy as np

NC_N = 8
B, T, DD, E, H = 32, 128, 2048, 1024, 2048
HS = H // NC_N            # 256 hidden rows per core
G3 = 3 * HS               # 768 gate rows per core
C, KQ, CD = 22, 1024, 128
KS = KQ // NC_N           # 128 queue slots per core
RP = B * T // NC_N        # 512 rows per core in phase A
BPC = B // NC_N           # 4 batches per core
MOM = 0.999
MASK_RATIO = 0.25
EPS = 1e-5

_CACHE = {}


def _build(t_steps=T, rp=RP):
    import concourse.bass as bass
    import concourse.mybir as mybir
    import concourse.tile as tile
    from concourse import bacc
    from concourse.masks import make_identity
    from contextlib import ExitStack

    fp32 = mybir.dt.float32
    f32r = mybir.dt.float32r
    AF = mybir.ActivationFunctionType
    OP = mybir.AluOpType
    RG = [list(range(NC_N))]

    nc = bacc.Bacc("TRN2", target_bir_lowering=False, debug=False,
                   num_devices=NC_N)

    def r(ap):
        return ap.bitcast(f32r)

    def bcast(ap, p=128):
        return bass.AP(tensor=ap.tensor, offset=ap.offset,
                       ap=[[0, p]] + list(ap.ap))

    rows_total = (rp * NC_N)
    n_rt = rp // 128           # row tiles per core (4)
    n_m = rows_total // 128    # row tiles across all cores (32)
    n_dc = 2 * DD // 128       # 32 contraction chunks for layer1
    n_ec = E // 128            # 8
    n_kc = H // 128            # 16

    # ---------------- inputs ----------------
    def inp(name, shape):
        return nc.dram_tensor(name, list(shape), fp32, kind="ExternalInput")

    rgb_rows = inp("rgb_rows", [rp, DD])
    flow_rows = inp("flow_rows", [rp, DD])
    mask_rows = inp("mask_rows", [rp])
    w1 = {"q": inp("w1_q", [2 * DD, E]), "k": inp("w1_k", [2 * DD, E])}
    b1 = {"q": inp("b1_q", [E]), "k": inp("b1_k", [E])}
    g_ = {"q": inp("g_q", [E]), "k": inp("g_k", [E])}
    be_ = {"q": inp("be_q", [E]), "k": inp("be_k", [E])}
    wih = {e: [inp(f"wih_{e}_{g}", [HS, E]) for g in range(3)]
           for e in ("q", "k")}
    whh = {e: [inp(f"whh_{e}_{g}", [HS, H]) for g in range(3)]
           for e in ("q", "k")}
    biasfold = {"q": inp("biasfold_q", [G3]), "k": inp("biasfold_k", [G3])}
    bhhn = {"q": inp("bhhn_q", [HS]), "k": inp("bhhn_k", [HS])}
    wq_in = inp("wq", [H, CD])
    bq_in = inp("bq", [CD])
    queues_in = inp("queues_j", [C, CD, KS])
    s_in = inp("S_j", [C, B, KS])
    mnot_in = inp("Mnot_j", [C, KS])

    q_cls_out = nc.dram_tensor("q_cls", [B, CD], fp32, kind="ExternalOutput")
    k_cls_out = nc.dram_tensor("k_cls", [B, CD], fp32, kind="ExternalOutput")
    newq_out = nc.dram_tensor("newq", [C, CD, KS], fp32,
                              kind="ExternalOutput")

    ENCS = ("q", "k")

    with tile.TileContext(nc) as tc, ExitStack() as ctx:
        const = ctx.enter_context(tc.tile_pool(name="const", bufs=1))
        dramp = ctx.enter_context(tc.tile_pool(name="dramp", bufs=1,
                                               space="DRAM"))
        agd = ctx.enter_context(tc.tile_pool(name="agd", bufs=4,
                                             space="DRAM"))
        bh = ctx.enter_context(tc.tile_pool(name="bh", bufs=2))
        acst2 = ctx.enter_context(tc.tile_pool(name="acst2", bufs=1))

        # ---------------- persistent constants (small) ----------------
        ident = const.tile([128, 128], fp32, tag="ident", name="ident")
        make_identity(nc, ident[:, :])
        ones_src = const.tile([1, 128], fp32, tag="ones_src", name="ones_src")
        nc.vector.memset(ones_src[:, :], 1.0)
        ones_row = const.tile([1, 128], fp32, tag="ones_row", name="ones_row")
        nc.vector.tensor_copy(out=ones_row[:, :].bitcast(f32r),
                              in_=ones_src[:, :])
        bhhnb = {}
        for e in ENCS:
            bhhnb[e] = const.tile([32, HS], fp32, tag=f"bhhnb_{e}",
                                  name=f"bhhnb_{e}")
            nc.sync.dma_start(out=bhhnb[e][:, :],
                              in_=bcast(bhhn[e].ap(), p=32))
        bq_row = const.tile([1, CD], fp32, tag="bq_row", name="bq_row")
        nc.sync.dma_start(out=bq_row[:, :].bitcast(f32r),
                          in_=bq_in.ap().bitcast(f32r))
        wq_sb = const.tile([128, n_kc, CD], fp32, tag="wq_sb", name="wq_sb")
        nc.sync.dma_start(
            out=wq_sb[:, :, :].bitcast(f32r),
            in_=wq_in.ap().rearrange("(kc p) c -> p kc c",
                                     p=128).bitcast(f32r))

        # DRAM intermediates
        xcat_dram = dramp.tile([2 * DD, rp], fp32, tag="xcat_dram",
                               name="xcat_dram")
        xi_dram = {e: dramp.tile([rows_total, G3], fp32, tag=f"xi_{e}",
                                 name=f"xi_{e}") for e in ENCS}
        xall = {e: dramp.tile([NC_N, E, rp], fp32, tag=f"xall_{e}",
                              name=f"xall_{e}", addr_space="Shared")
                for e in ENCS}

        # ================= PHASE A =================
        with ExitStack() as actx:
            acst = actx.enter_context(tc.tile_pool(name="acst", bufs=1))
            tmp = actx.enter_context(tc.tile_pool(name="tmpA", bufs=2))
            lnp = actx.enter_context(tc.tile_pool(name="lnp", bufs=4))
            a2 = actx.enter_context(tc.tile_pool(name="a2", bufs=2))
            xl = actx.enter_context(tc.tile_pool(name="xl", bufs=4))
            wprep = actx.enter_context(tc.tile_pool(name="wprep", bufs=1))
            xsb = actx.enter_context(tc.tile_pool(name="xsb", bufs=1))

            # phase-A constants
            mask_b = acst.tile([128, rp], fp32, tag="mask_b", name="mask_b")
            nc.sync.dma_start(out=mask_b[:, :], in_=bcast(mask_rows.ap()))
            epst = acst.tile([128, 1], fp32, tag="epst", name="epst")
            nc.vector.memset(epst[:, :], EPS)
            gb, beb, b1row, bfrow = {}, {}, {}, {}
            for e in ENCS:
                gb[e] = acst.tile([128, E], fp32, tag=f"gb_{e}",
                                  name=f"gb_{e}")
                nc.sync.dma_start(out=gb[e][:, :], in_=bcast(g_[e].ap()))
                beb[e] = acst.tile([128, E], fp32, tag=f"beb_{e}",
                                   name=f"beb_{e}")
                nc.sync.dma_start(out=beb[e][:, :], in_=bcast(be_[e].ap()))
                b1row[e] = acst.tile([1, E], fp32, tag=f"b1row_{e}",
                                     name=f"b1row_{e}")
                nc.sync.dma_start(out=b1row[e][:, :].bitcast(f32r),
                                  in_=b1[e].ap().bitcast(f32r))
                bfrow[e] = acst2.tile([1, G3], fp32, tag=f"bfrow_{e}",
                                      name=f"bfrow_{e}")
                nc.sync.dma_start(out=bfrow[e][:, :].bitcast(f32r),
                                  in_=biasfold[e].ap().bitcast(f32r))

            # ---- A1: transpose inputs into xcat_dram [2D, rp] ----
            with tc.tile_pool(name="tpsA1", bufs=4, space="PSUM") as tps1:
                for kc in range(n_dc):
                    src = rgb_rows if kc < DD // 128 else flow_rows
                    dcol = (kc % (DD // 128)) * 128
                    slab = tmp.tile([128, n_rt, 128], fp32, tag="slab",
                                    name="slab")
                    nc.sync.dma_start(
                        out=slab[:, :, :],
                        in_=src[:, dcol:dcol + 128]
                        .rearrange("(rt p) d -> p rt d", p=128))
                    xc_sb = tmp.tile([128, rp], fp32, tag="xc_sb",
                                     name="xc_sb")
                    for rt in range(n_rt):
                        ps = tps1.tile([128, 128], fp32, tag="tpsa",
                                       name="tpsa")
                        nc.tensor.transpose(ps[:, :], slab[:, rt, :],
                                            ident[:, :])
                        nc.vector.tensor_copy(
                            out=xc_sb[:, rt * 128:(rt + 1) * 128],
                            in_=ps[:, :])
                    nc.sync.dma_start(
                        out=xcat_dram[kc * 128:(kc + 1) * 128, :],
                        in_=xc_sb[:, :])

            # ---- layer1 + LN (x stays in SBUF), per encoder ----
            x_sb = {e: xsb.tile([128, n_rt, E], fp32, tag=f"x_sb_{e}",
                                name=f"x_sb_{e}") for e in ENCS}
            l1scope = ExitStack()
            l1ps = l1scope.enter_context(
                tc.tile_pool(name="l1ps", bufs=1, space="PSUM"))
            for e in ENCS:
                psl = [[l1ps.tile([128, 512], fp32, tag=f"l1_{rt}_{n}",
                                  name=f"l1_{rt}_{n}")
                        for n in range(2)] for rt in range(n_rt)]
                for kc in range(n_dc):
                    xck = tmp.tile([128, rp], fp32, tag="xck", name="xck")
                    nc.sync.dma_start(
                        out=xck[:, :].bitcast(f32r),
                        in_=xcat_dram[kc * 128:(kc + 1) * 128,
                                      :].bitcast(f32r))
                    if e == "q":
                        xmm = tmp.tile([128, rp], fp32, tag="xmm",
                                       name="xmm")
                        nc.vector.tensor_mul(xmm[:, :].bitcast(f32r),
                                             xck[:, :], mask_b[:, :])
                        src_chunk = xmm
                    else:
                        src_chunk = xck
                    w1row = tmp.tile([128, E], fp32, tag="w1row",
                                     name="w1row")
                    nc.sync.dma_start(
                        out=w1row[:, :].bitcast(f32r),
                        in_=w1["q"][kc * 128:(kc + 1) * 128,
                                    :].bitcast(f32r))
                    if e == "k":
                        w1k = tmp.tile([128, E], fp32, tag="w1k", name="w1k")
                        nc.sync.dma_start(
                            out=w1k[:, :],
                            in_=w1["k"][kc * 128:(kc + 1) * 128, :])
                        w1c = tmp.tile([128, E], fp32, tag="w1c", name="w1c")
                        nc.vector.tensor_scalar_mul(w1c[:, :].bitcast(f32r),
                                                    w1k[:, :], MOM)
                        nc.vector.scalar_tensor_tensor(
                            out=w1c[:, :].bitcast(f32r), in0=w1row[:, :],
                            scalar=1.0 - MOM,
                            in1=w1c[:, :], op0=OP.mult, op1=OP.add)
                        wrow = w1c
                    else:
                        wrow = w1row
                    for rt in range(n_rt):
                        for n in range(2):
                            nc.tensor.matmul(
                                psl[rt][n][:, :],
                                r(src_chunk[:, rt * 128:(rt + 1) * 128]),
                                r(wrow[:, n * 512:(n + 1) * 512]),
                                start=(kc == 0), stop=False)
                for rt in range(n_rt):
                    for n in range(2):
                        nc.tensor.matmul(
                            psl[rt][n][:, :], r(ones_row[:, 0:128]),
                            r(b1row[e][:, n * 512:(n + 1) * 512]),
                            start=False, stop=True)
                # LN + affine into x_sb (relu folded into transpose copy)
                for rt in range(n_rt):
                    stats = lnp.tile([128, 2, 6], fp32, tag="stats",
                                     name="stats")
                    nc.vector.bn_stats(out=stats[:, 0, :],
                                       in_=psl[rt][0][:, :])
                    nc.vector.bn_stats(out=stats[:, 1, :],
                                       in_=psl[rt][1][:, :])
                    mv = lnp.tile([128, 2], fp32, tag="mv", name="mv")
                    nc.vector.bn_aggr(out=mv[:, :], in_=stats[:, :, :])
                    rstd = lnp.tile([128, 1], fp32, tag="rstd", name="rstd")
                    nc.scalar.activation(out=rstd[:, :], in_=mv[:, 1:2],
                                         func=AF.Sqrt, bias=epst[:, :],
                                         scale=1.0)
                    nc.vector.reciprocal(out=rstd[:, :], in_=rstd[:, :])
                    nmr = lnp.tile([128, 1], fp32, tag="nmr", name="nmr")
                    nc.vector.scalar_tensor_tensor(
                        out=nmr[:, :], in0=mv[:, 0:1], scalar=-1.0,
                        in1=rstd[:, :], op0=OP.mult, op1=OP.mult)
                    t1 = a2.tile([128, E], fp32, tag="t1", name="t1")
                    for n in range(2):
                        nc.scalar.activation(
                            out=t1[:, n * 512:(n + 1) * 512],
                            in_=psl[rt][n][:, :], func=AF.Identity,
                            bias=nmr[:, :], scale=rstd[:, :])
                    nc.vector.tensor_mul(t1[:, :], t1[:, :], gb[e][:, :])
                    nc.vector.tensor_add(x_sb[e][:, rt, :], t1[:, :],
                                         beb[e][:, :])
            l1scope.close()

            # ---- x^T transposes (relu folded) + AllGather, per encoder ----
            tps2 = actx.enter_context(tc.tile_pool(name="tps2", bufs=4,
                                                   space="PSUM"))
            for e in ENCS:
                agi = agd.tile([E, rp], fp32, tag="agx_in", name="agx_in")
                for rt in range(n_rt):
                    xtp = a2.tile([128, n_ec, 128], fp32, tag="xtp",
                                  name="xtp")
                    for ec in range(n_ec):
                        ps = tps2.tile([128, 128], fp32, tag="tpsa2",
                                       name="tpsa2")
                        nc.tensor.transpose(
                            ps[:, :],
                            x_sb[e][:, rt, ec * 128:(ec + 1) * 128],
                            ident[:, :])
                        nc.vector.tensor_scalar_max(xtp[:, ec, :],
                                                    ps[:, :], 0.0)
                    nc.sync.dma_start(
                        out=agi[:, rt * 128:(rt + 1) * 128]
                        .rearrange("(ec p) c -> p ec c", p=128),
                        in_=xtp[:, :, :])
                nc.gpsimd.collective_compute(
                    "AllGather", OP.bypass, replica_groups=RG,
                    ins=[agi[:, :].opt()], outs=[xall[e][:, :, :].opt()])


        # ================= whh^T / wih^T prep + xi GEMMs =================
        # (whh prep first: fills the x-AllGather latency with useful work)
        wscope = ctx.enter_context(tc.tile_pool(name="wscope", bufs=1))
        whhT = {e: wscope.tile([128, n_kc, G3], fp32, tag=f"whhT_{e}",
                               name=f"whhT_{e}") for e in ENCS}
        with ExitStack() as pctx:
            wtmp = pctx.enter_context(tc.tile_pool(name="wtmp", bufs=1))
            ptps = pctx.enter_context(tc.tile_pool(name="ptps", bufs=4,
                                                   space="PSUM"))
            wprep = pctx.enter_context(tc.tile_pool(name="wprep", bufs=1))
            xl = pctx.enter_context(tc.tile_pool(name="xl", bufs=4))
            xa2 = pctx.enter_context(tc.tile_pool(name="xa2", bufs=2))
            xip = pctx.enter_context(tc.tile_pool(name="xip", bufs=2,
                                                  space="PSUM"))

            def prep_transposed(dst, srcs, g, rt2, nchunk, is_k):
                ncols = srcs["q"][g].shape[1]
                wt = wtmp.tile([128, ncols], fp32, tag="wr_a", name="wr_a")
                nc.sync.dma_start(
                    out=wt[:, :],
                    in_=srcs["q"][g][rt2 * 128:(rt2 + 1) * 128, :])
                if is_k:
                    wtk = wtmp.tile([128, ncols], fp32, tag="wr_b",
                                    name="wr_b")
                    nc.sync.dma_start(
                        out=wtk[:, :],
                        in_=srcs["k"][g][rt2 * 128:(rt2 + 1) * 128, :])
                    wtc = wtmp.tile([128, ncols], fp32, tag="wr_c",
                                    name="wr_c")
                    nc.vector.tensor_scalar_mul(wtc[:, :], wtk[:, :], MOM)
                    nc.vector.scalar_tensor_tensor(
                        out=wtc[:, :], in0=wt[:, :], scalar=1.0 - MOM,
                        in1=wtc[:, :], op0=OP.mult, op1=OP.add)
                    wt = wtc
                for cc2 in range(nchunk):
                    ps = ptps.tile([128, 128], fp32, tag="ptpsa",
                                   name="ptpsa")
                    nc.tensor.transpose(
                        ps[:, :], wt[:, cc2 * 128:(cc2 + 1) * 128],
                        ident[:, :])
                    nc.vector.tensor_copy(
                        out=dst[:, cc2,
                                g * HS + rt2 * 128:
                                g * HS + (rt2 + 1) * 128].bitcast(f32r),
                        in_=ps[:, :])

            for e in ENCS:
                for g in range(3):
                    for rt2 in range(HS // 128):
                        prep_transposed(whhT[e], whh, g, rt2, n_kc,
                                        e == "k")

            for e in ENCS:
                wihT = wprep.tile([128, n_ec, G3], fp32, tag="wihT",
                                  name="wihT")
                for g in range(3):
                    for rt2 in range(HS // 128):
                        prep_transposed(wihT, wih, g, rt2, n_ec, e == "k")
                for m in range(n_m):
                    jb, sub = divmod(m, n_rt)
                    psA = xip.tile([128, 512], fp32, tag="xipA", name="xipA")
                    psB = xip.tile([128, 256], fp32, tag="xipB", name="xipB")
                    for kc in range(n_ec):
                        lt = xl.tile([128, 128], fp32, tag="lt", name="lt")
                        nc.sync.dma_start(
                            out=lt[:, :].bitcast(f32r),
                            in_=xall[e][jb, kc * 128:(kc + 1) * 128,
                                        sub * 128:(sub + 1) * 128]
                            .bitcast(f32r))
                        nc.tensor.matmul(psA[:, :], r(lt[:, :]),
                                         r(wihT[:, kc, 0:512]),
                                         start=(kc == 0), stop=False)
                        nc.tensor.matmul(psB[:, :], r(lt[:, :]),
                                         r(wihT[:, kc, 512:G3]),
                                         start=(kc == 0), stop=False)
                    nc.tensor.matmul(psA[:, :], r(ones_row[:, 0:128]),
                                     r(bfrow[e][:, 0:512]),
                                     start=False, stop=True)
                    nc.tensor.matmul(psB[:, :], r(ones_row[:, 0:128]),
                                     r(bfrow[e][:, 512:G3]),
                                     start=False, stop=True)
                    xi_sb = xa2.tile([128, G3], fp32, tag="xi_sb",
                                     name="xi_sb")
                    nc.vector.tensor_copy(out=xi_sb[:, 0:512], in_=psA[:, :])
                    nc.vector.tensor_copy(out=xi_sb[:, 512:G3],
                                          in_=psB[:, :])
                    nc.sync.dma_start(
                        out=xi_dram[e][m * 128:(m + 1) * 128, :],
                        in_=xi_sb[:, :])

        # ================= PHASE B: recurrence =================
        with ExitStack() as bctx:
            bxi = bctx.enter_context(tc.tile_pool(name="bxi", bufs=2))
            bgate = bctx.enter_context(tc.tile_pool(name="bgate", bufs=2))
            bps = bctx.enter_context(tc.tile_pool(name="bps", bufs=1,
                                                  space="PSUM"))
            btps = bctx.enter_context(tc.tile_pool(name="btps", bufs=2,
                                                   space="PSUM"))
            hT = {e: None for e in ENCS}
            h_prev = {e: None for e in ENCS}
            xi_r = {e: xi_dram[e][:, :].rearrange("(b t) g -> t b g",
                                                  t=t_steps)
                    for e in ENCS}
            for t in range(t_steps):
                # combined AG input: [q slice (256); k slice (256)] x 32
                agi = agd.tile([2 * HS, 32], fp32, tag="agh_in",
                               name="agh_in")
                rzs = {}
                xis = {}
                # 1) r/z matmuls + sigmoid for both encoders first
                for e in ENCS:
                    xi_t = bxi.tile([32, G3], fp32, tag=f"xi_t_{e}",
                                    name=f"xi_t_{e}")
                    nc.sync.dma_start(out=xi_t[:, :], in_=xi_r[e][t])
                    xis[e] = xi_t
                    trz = bgate.tile([32, 512], fp32, tag=f"trz_{e}",
                                     name=f"trz_{e}")
                    if t > 0:
                        ps_rz = bps.tile([32, 512], fp32, tag=f"psrz_{e}",
                                         name=f"psrz_{e}")
                        for kc in range(n_kc):
                            nc.tensor.matmul(
                                ps_rz[:, :], r(hT[e][:, kc, :]),
                                r(whhT[e][:, kc, 0:512]),
                                start=(kc == 0), stop=(kc == n_kc - 1))
                        nc.vector.tensor_add(trz[:, :], ps_rz[:, :],
                                             xi_t[:, 0:512])
                    else:
                        nc.vector.tensor_copy(out=trz[:, :],
                                              in_=xi_t[:, 0:512])
                    rz = bgate.tile([32, 512], fp32, tag=f"rz_{e}",
                                    name=f"rz_{e}")
                    nc.scalar.activation(out=rz[:, :], in_=trz[:, :],
                                         func=AF.Sigmoid)
                    rzs[e] = rz
                # 2) n matmuls + gate tail + transpose, per encoder
                for ei, e in enumerate(ENCS):
                    xi_t = xis[e]
                    rz = rzs[e]
                    tn = bgate.tile([32, HS], fp32, tag=f"tn_{e}",
                                    name=f"tn_{e}")
                    if t > 0:
                        ps_n = bps.tile([32, 256], fp32, tag=f"psn_{e}",
                                        name=f"psn_{e}")
                        for kc in range(n_kc):
                            nc.tensor.matmul(
                                ps_n[:, :], r(hT[e][:, kc, :]),
                                r(whhT[e][:, kc, 512:G3]),
                                start=(kc == 0), stop=(kc == n_kc - 1))
                        nc.vector.tensor_add(tn[:, :], ps_n[:, :],
                                             bhhnb[e][:, :])
                    else:
                        nc.vector.tensor_copy(out=tn[:, :],
                                              in_=bhhnb[e][:, :])
                    nc.vector.tensor_mul(tn[:, :], rz[:, 0:256], tn[:, :])
                    nc.vector.tensor_add(tn[:, :], tn[:, :], xi_t[:, 512:G3])
                    nn_t = bgate.tile([32, HS], fp32, tag=f"nn_{e}",
                                      name=f"nn_{e}")
                    nc.scalar.activation(out=nn_t[:, :], in_=tn[:, :],
                                         func=AF.Tanh)
                    d_t = bgate.tile([32, HS], fp32, tag=f"d_{e}",
                                     name=f"d_{e}")
                    if t > 0:
                        nc.vector.tensor_sub(d_t[:, :], h_prev[e][:, :],
                                             nn_t[:, :])
                    else:
                        nc.vector.tensor_scalar_mul(d_t[:, :], nn_t[:, :],
                                                    -1.0)
                    nc.vector.tensor_mul(d_t[:, :], rz[:, 256:512],
                                         d_t[:, :])
                    h_new = bh.tile([32, HS], fp32, tag=f"hnew_{e}",
                                    name=f"hnew_{e}")
                    nc.vector.tensor_add(h_new[:, :], nn_t[:, :], d_t[:, :])
                    h_prev[e] = h_new
                    hts = bgate.tile([128, HS // 128, 32], fp32,
                                     tag=f"hts_{e}", name=f"hts_{e}")
                    for hf in range(HS // 128):
                        pst = btps.tile([128, 32], fp32, tag="pst",
                                        name="pst")
                        nc.tensor.transpose(
                            pst[:, :], h_new[:, hf * 128:(hf + 1) * 128],
                            ident[0:32, 0:32])
                        nc.vector.tensor_copy(out=hts[:, hf, :],
                                              in_=pst[:, :])
                    nc.sync.dma_start(
                        out=agi[ei * HS:(ei + 1) * HS, :]
                        .rearrange("(hf p) c -> p hf c", p=128),
                        in_=hts[:, :, :])
                # 3) one AllGather for both encoders
                ago = agd.tile([NC_N * 2 * HS, 32], fp32, tag="agh_out",
                               name="agh_out", addr_space="Shared")
                nc.gpsimd.collective_compute(
                    "AllGather", OP.bypass, replica_groups=RG,
                    ins=[agi[:, :].opt()], outs=[ago[:, :].opt()])
                for ei, e in enumerate(ENCS):
                    hT_new = bh.tile([128, n_kc, 32], fp32, tag=f"hT_{e}",
                                     name=f"hT_{e}")
                    for hf in range(HS // 128):
                        src_ap = bass.AP(
                            tensor=ago.tensor,
                            offset=(ago.offset + ei * HS * 32
                                    + hf * 128 * 32),
                            ap=[[32, 128], [2 * HS * 32, NC_N], [1, 32]])
                        out_ap = bass.AP(
                            tensor=hT_new.tensor,
                            offset=hT_new.offset + hf * 32,
                            ap=[[n_kc * 32, 128], [2 * 32, NC_N], [1, 32]])
                        nc.sync.dma_start(
                            out=out_ap.bitcast(f32r),
                            in_=src_ap.bitcast(f32r))
                    hT[e] = hT_new

        # ================= PHASE C: head + queues =================
        with ExitStack() as cctx:
            cp = cctx.enter_context(tc.tile_pool(name="cp", bufs=2))
            cps = cctx.enter_context(tc.tile_pool(name="cps", bufs=2,
                                                  space="PSUM"))
            qt = cctx.enter_context(tc.tile_pool(name="qt", bufs=3))
            qconst = cctx.enter_context(tc.tile_pool(name="qconst", bufs=1))

            s_sb = qconst.tile([B, C, KS], fp32, tag="s_sb", name="s_sb")
            nc.sync.dma_start(
                out=s_sb[:, :, :].bitcast(f32r),
                in_=s_in.ap().rearrange("c b k -> b c k").bitcast(f32r))
            mnot_b = qconst.tile([128, C, KS], fp32, tag="mnot_b",
                                 name="mnot_b")
            nc.sync.dma_start(out=mnot_b[:, :, :], in_=bcast(mnot_in.ap()))
            qsb = qconst.tile([128, C, KS], fp32, tag="qsb", name="qsb")
            nc.sync.dma_start(out=qsb[:, :, :],
                              in_=queues_in.ap().rearrange("c p k -> p c k"))

            cls_sb = {}
            for e in ENCS:
                featT = cp.tile([128, n_kc, 32], fp32, tag=f"featT_{e}",
                                name=f"featT_{e}")
                nc.scalar.activation(out=featT[:, :, :].bitcast(f32r),
                                     in_=hT[e][:, :, :], func=AF.Relu)
                ps_cls = cps.tile([32, CD], fp32, tag=f"pscls_{e}",
                                  name=f"pscls_{e}")
                for kc in range(n_kc):
                    nc.tensor.matmul(ps_cls[:, :], r(featT[:, kc, :]),
                                     r(wq_sb[:, kc, :]),
                                     start=(kc == 0), stop=False)
                nc.tensor.matmul(ps_cls[:, :], r(ones_row[:, 0:32]),
                                 r(bq_row[:, :]), start=False, stop=True)
                sq = cp.tile([32, CD], fp32, tag=f"sq_{e}", name=f"sq_{e}")
                ssum = cp.tile([32, 1], fp32, tag=f"ssum_{e}",
                               name=f"ssum_{e}")
                nc.scalar.activation(out=sq[:, :], in_=ps_cls[:, :],
                                     func=AF.Square, accum_out=ssum[:, :])
                rn = cp.tile([32, 1], fp32, tag=f"rn_{e}", name=f"rn_{e}")
                nc.scalar.activation(out=rn[:, :], in_=ssum[:, :],
                                     func=AF.Sqrt)
                nc.vector.reciprocal(out=rn[:, :], in_=rn[:, :])
                cls = cp.tile([32, CD], fp32, tag=f"cls_{e}",
                              name=f"cls_{e}")
                nc.scalar.activation(out=cls[:, :], in_=ps_cls[:, :],
                                     func=AF.Copy, scale=rn[:, :])
                cls_sb[e] = cls
                out_t = q_cls_out if e == "q" else k_cls_out
                nc.sync.dma_start(out=out_t[:, :], in_=cls[:, :])

            clsr = cp.tile([32, CD], fp32, tag="clsr", name="clsr")
            nc.vector.tensor_copy(out=clsr[:, :].bitcast(f32r),
                                  in_=cls_sb["k"][:, :])
            for c in range(C):
                psq = cps.tile([128, KS], fp32, tag="psq", name="psq")
                nc.tensor.matmul(psq[:, :], r(clsr[:, :]),
                                 r(s_sb[:, c, :]), start=True, stop=True)
                qn = qt.tile([128, KS], fp32, tag="qn", name="qn")
                nc.vector.tensor_mul(qn[:, :], qsb[:, c, :],
                                     mnot_b[:, c, :])
                nc.vector.tensor_add(qn[:, :], qn[:, :], psq[:, :])
                nc.sync.dma_start(out=newq_out[c, :, :], in_=qn[:, :])

    nc.compile()
    return nc


def _prep_inputs(inputs, t_steps=T, rp=RP):
    """Build the 8 per-core input maps from the full input dict."""
    f = {k: np.asarray(v) for k, v in inputs.items()}
    rgb = f["rgb"].astype(np.float32, copy=False)
    flow = f["flow"].astype(np.float32, copy=False)
    rand_mask = f["rand_mask"].astype(np.float32, copy=False)
    targets = f["targets"].astype(np.float32, copy=False)
    ptrs = f["ptrs"].astype(np.int64)
    queues = f["queues"].astype(np.float32, copy=False)

    b_ = rgb.shape[0]
    mask = (rand_mask[:, :, 0] > MASK_RATIO).astype(np.float32)
    mask[:, -1] = 1.0

    def mu(kp, qp):
        return (MOM * f[kp].astype(np.float64)
                + (1.0 - MOM) * f[qp].astype(np.float64)).astype(np.float32)

    b1_k = mu("b1_k", "b1_q")
    g_k = mu("g_k", "g_q")
    be_k = mu("be_k", "be_q")
    bih_k = mu("bih_k", "bih_q")
    bhh_k = mu("bhh_k", "bhh_q")
    bih_q = f["bih_q"].astype(np.float32, copy=False)
    bhh_q = f["bhh_q"].astype(np.float32, copy=False)

    sel = targets > 0.5
    pos = np.cumsum(sel, axis=0) - 1
    slot = (ptrs[None, :].astype(np.int64) + pos) % KQ
    cnt = sel.sum(0).astype(np.int64)
    new_ptrs = ((ptrs + cnt) % KQ).astype(np.int32)

    S = np.zeros((NC_N, C, b_, KS), np.float32)
    for bb in range(b_):
        for cc in range(C):
            if sel[bb, cc]:
                s = int(slot[bb, cc])
                S[s // KS, cc, bb, s % KS] = 1.0
    Mnot = 1.0 - S.sum(axis=2)  # [NC, C, KS]

    bpc = b_ // NC_N
    in_maps = []
    for j in range(NC_N):
        m = {
            "rgb_rows": np.ascontiguousarray(
                rgb[j * bpc:(j + 1) * bpc].reshape(rp, -1)),
            "flow_rows": np.ascontiguousarray(
                flow[j * bpc:(j + 1) * bpc].reshape(rp, -1)),
            "mask_rows": np.ascontiguousarray(
                mask[j * bpc:(j + 1) * bpc].reshape(rp)),
            "w1_q": f["w1_q"], "w1_k": f["w1_k"],
            "b1_q": f["b1_q"], "b1_k": b1_k,
            "g_q": f["g_q"], "g_k": g_k,
            "be_q": f["be_q"], "be_k": be_k,
            "wq": f["wq"], "bq": f["bq"],
            "queues_j": np.ascontiguousarray(
                queues[:, :, j * KS:(j + 1) * KS]),
            "S_j": np.ascontiguousarray(S[j]),
            "Mnot_j": np.ascontiguousarray(Mnot[j]),
        }
        hh = H
        for e in ("q", "k"):
            wihf = f[f"wih_{e}"]
            whhf = f[f"whh_{e}"]
            for g in range(3):
                sl = slice(g * hh + j * HS, g * hh + (j + 1) * HS)
                m[f"wih_{e}_{g}"] = np.ascontiguousarray(wihf[sl])
                m[f"whh_{e}_{g}"] = np.ascontiguousarray(whhf[sl])
        for e, bihv, bhhv in (("q", bih_q, bhh_q), ("k", bih_k, bhh_k)):
            bf = np.empty(G3, np.float32)
            for g in range(3):
                sl = slice(g * hh + j * HS, g * hh + (j + 1) * HS)
                bf[g * HS:(g + 1) * HS] = bihv[sl]
                if g < 2:  # bhh for r,z folded; n-gate bhh applied in-step
                    bf[g * HS:(g + 1) * HS] += bhhv[sl]
            m[f"biasfold_{e}"] = bf
            m[f"bhhn_{e}"] = np.ascontiguousarray(
                bhhv[2 * hh + j * HS: 2 * hh + (j + 1) * HS])
        in_maps.append(m)
    return in_maps, new_ptrs


def _assemble(results, new_ptrs):
    q_cls = np.asarray(results[0]["q_cls"])
    k_cls = np.asarray(results[0]["k_cls"])
    new_queues = np.concatenate(
        [np.asarray(results[j]["newq"]) for j in range(NC_N)], axis=2)
    return q_cls, k_cls, new_queues, new_ptrs


def kernel(**inputs):
    from concourse import bass_utils
    if "nc" not in _CACHE:
        _CACHE["nc"] = _build()
    nc = _CACHE["nc"]
    in_maps, new_ptrs = _prep_inputs(inputs)
    res = bass_utils.run_bass_kernel_spmd(nc, in_maps,
                                          core_ids=list(range(NC_N)))
    return _assemble(res.results, new_ptrs)


# revision 20
# speedup vs baseline: 1.0294x; 1.0014x over previous
"""Trainium2 Bass kernel for nn_ContrastiveMROADMultiQueue.

Contract: kernel(**inputs) takes FULL unsharded inputs (as in
reference.setup_inputs()) and returns the full outputs
(q_cls, k_cls, new_queues, new_ptrs).

Sharding strategy (8 NeuronCores, SPMD single graph, per-core data):
  Phase A: row-parallel (batch*time/8 = 512 rows per core) layer1 + LayerNorm
           + ReLU for both encoders (key weights momentum-combined on device);
           AllGather of x^T; then column-parallel xi = x @ wih^T GEMM where
           each core computes exactly the 3H-slice (768 cols = r|z|n x 256)
           it needs for its share of the recurrence.
  Phase B: GRU recurrence with the hidden dim split 8 ways (256 rows of each
           gate per core). Query and key encoders are interleaved per step so
           each per-step AllGather of h-slices hides behind the other
           encoder's matmul. h is kept in transposed [H, B] layout end-to-end.
  Phase C: replicated head (relu -> wq -> l2norm) and a K-split (128 queue
           slots per core) scatter of k_cls into the per-class queues done
           with static-shape matmuls against host-built selection matrices.
"""

import numpy as np

NC_N = 8
B, T, DD, E, H = 32, 128, 2048, 1024, 2048
HS = H // NC_N            # 256 hidden rows per core
G3 = 3 * HS               # 768 gate rows per core
C, KQ, CD = 22, 1024, 128
KS = KQ // NC_N           # 128 queue slots per core
RP = B * T // NC_N        # 512 rows per core in phase A
BPC = B // NC_N           # 4 batches per core
MOM = 0.999
MASK_RATIO = 0.25
EPS = 1e-5

_CACHE = {}


def _build(t_steps=T, rp=RP):
    import concourse.bass as bass
    import concourse.mybir as mybir
    import concourse.tile as tile
    from concourse import bacc
    from concourse.masks import make_identity
    from contextlib import ExitStack

    fp32 = mybir.dt.float32
    f32r = mybir.dt.float32r
    AF = mybir.ActivationFunctionType
    OP = mybir.AluOpType
    RG = [list(range(NC_N))]

    nc = bacc.Bacc("TRN2", target_bir_lowering=False, debug=False,
                   num_devices=NC_N)

    def r(ap):
        return ap.bitcast(f32r)

    def bcast(ap, p=128):
        return bass.AP(tensor=ap.tensor, offset=ap.offset,
                       ap=[[0, p]] + list(ap.ap))

    _dma_i = [0]

    def dma(out=None, in_=None):
        eng = (nc.sync, nc.scalar)[_dma_i[0] & 1]
        _dma_i[0] += 1
        eng.dma_start(out=out, in_=in_)

    rows_total = (rp * NC_N)
    n_rt = rp // 128           # row tiles per core (4)
    n_m = rows_total // 128    # row tiles across all cores (32)
    n_dc = 2 * DD // 128       # 32 contraction chunks for layer1
    n_ec = E // 128            # 8
    n_kc = H // 128            # 16

    # ---------------- inputs ----------------
    def inp(name, shape):
        return nc.dram_tensor(name, list(shape), fp32, kind="ExternalInput")

    rgb_rows = inp("rgb_rows", [rp, DD])
    flow_rows = inp("flow_rows", [rp, DD])
    mask_rows = inp("mask_rows", [rp])
    w1 = {"q": inp("w1_q", [2 * DD, E]), "k": inp("w1_k", [2 * DD, E])}
    b1 = {"q": inp("b1_q", [E]), "k": inp("b1_k", [E])}
    g_ = {"q": inp("g_q", [E]), "k": inp("g_k", [E])}
    be_ = {"q": inp("be_q", [E]), "k": inp("be_k", [E])}
    wih = {e: [inp(f"wih_{e}_{g}", [HS, E]) for g in range(3)]
           for e in ("q", "k")}
    whh = {e: [inp(f"whh_{e}_{g}", [HS, H]) for g in range(3)]
           for e in ("q", "k")}
    biasfold = {"q": inp("biasfold_q", [G3]), "k": inp("biasfold_k", [G3])}
    bhhn = {"q": inp("bhhn_q", [HS]), "k": inp("bhhn_k", [HS])}
    wq_in = inp("wq", [H, CD])
    bq_in = inp("bq", [CD])
    queues_in = inp("queues_j", [C, CD, KS])
    s_in = inp("S_j", [C, B, KS])
    mnot_in = inp("Mnot_j", [C, KS])

    q_cls_out = nc.dram_tensor("q_cls", [B, CD], fp32, kind="ExternalOutput")
    k_cls_out = nc.dram_tensor("k_cls", [B, CD], fp32, kind="ExternalOutput")
    newq_out = nc.dram_tensor("newq", [C, CD, KS], fp32,
                              kind="ExternalOutput")

    ENCS = ("q", "k")

    with tile.TileContext(nc) as tc, ExitStack() as ctx:
        const = ctx.enter_context(tc.tile_pool(name="const", bufs=1))
        dramp = ctx.enter_context(tc.tile_pool(name="dramp", bufs=1,
                                               space="DRAM"))
        agd = ctx.enter_context(tc.tile_pool(name="agd", bufs=4,
                                             space="DRAM"))
        bh = ctx.enter_context(tc.tile_pool(name="bh", bufs=2))
        acst2 = ctx.enter_context(tc.tile_pool(name="acst2", bufs=1))

        # ---------------- persistent constants (small) ----------------
        ident = const.tile([128, 128], fp32, tag="ident", name="ident")
        make_identity(nc, ident[:, :])
        ones_src = const.tile([1, 128], fp32, tag="ones_src", name="ones_src")
        nc.vector.memset(ones_src[:, :], 1.0)
        ones_row = const.tile([1, 128], fp32, tag="ones_row", name="ones_row")
        nc.vector.tensor_copy(out=ones_row[:, :].bitcast(f32r),
                              in_=ones_src[:, :])
        bhhnb = {}
        for e in ENCS:
            bhhnb[e] = const.tile([32, HS], fp32, tag=f"bhhnb_{e}",
                                  name=f"bhhnb_{e}")
            dma(out=bhhnb[e][:, :],
                              in_=bcast(bhhn[e].ap(), p=32))
        bq_row = const.tile([1, CD], fp32, tag="bq_row", name="bq_row")
        dma(out=bq_row[:, :].bitcast(f32r),
                          in_=bq_in.ap().bitcast(f32r))
        wq_sb = const.tile([128, n_kc, CD], fp32, tag="wq_sb", name="wq_sb")
        dma(
            out=wq_sb[:, :, :].bitcast(f32r),
            in_=wq_in.ap().rearrange("(kc p) c -> p kc c",
                                     p=128).bitcast(f32r))

        # DRAM intermediates
        xcat_dram = dramp.tile([2 * DD, rp], fp32, tag="xcat_dram",
                               name="xcat_dram")
        xi_dram = {e: dramp.tile([rows_total, G3], fp32, tag=f"xi_{e}",
                                 name=f"xi_{e}") for e in ENCS}
        xall = {e: dramp.tile([NC_N, E, rp], fp32, tag=f"xall_{e}",
                              name=f"xall_{e}", addr_space="Shared")
                for e in ENCS}

        # ================= PHASE A =================
        with ExitStack() as actx:
            acst = actx.enter_context(tc.tile_pool(name="acst", bufs=1))
            tmp = actx.enter_context(tc.tile_pool(name="tmpA", bufs=2))
            lnp = actx.enter_context(tc.tile_pool(name="lnp", bufs=4))
            a2 = actx.enter_context(tc.tile_pool(name="a2", bufs=2))
            xl = actx.enter_context(tc.tile_pool(name="xl", bufs=4))
            wprep = actx.enter_context(tc.tile_pool(name="wprep", bufs=1))
            xsb = actx.enter_context(tc.tile_pool(name="xsb", bufs=1))

            # phase-A constants
            mask_b = acst.tile([128, rp], fp32, tag="mask_b", name="mask_b")
            dma(out=mask_b[:, :], in_=bcast(mask_rows.ap()))
            epst = acst.tile([128, 1], fp32, tag="epst", name="epst")
            nc.vector.memset(epst[:, :], EPS)
            gb, beb, b1row, bfrow = {}, {}, {}, {}
            for e in ENCS:
                gb[e] = acst.tile([128, E], fp32, tag=f"gb_{e}",
                                  name=f"gb_{e}")
                dma(out=gb[e][:, :], in_=bcast(g_[e].ap()))
                beb[e] = acst.tile([128, E], fp32, tag=f"beb_{e}",
                                   name=f"beb_{e}")
                dma(out=beb[e][:, :], in_=bcast(be_[e].ap()))
                b1row[e] = acst.tile([1, E], fp32, tag=f"b1row_{e}",
                                     name=f"b1row_{e}")
                dma(out=b1row[e][:, :].bitcast(f32r),
                                  in_=b1[e].ap().bitcast(f32r))
                bfrow[e] = acst2.tile([1, G3], fp32, tag=f"bfrow_{e}",
                                      name=f"bfrow_{e}")
                dma(out=bfrow[e][:, :].bitcast(f32r),
                                  in_=biasfold[e].ap().bitcast(f32r))

            # ---- A1: transpose inputs into xcat_dram [2D, rp] ----
            with tc.tile_pool(name="tpsA1", bufs=4, space="PSUM") as tps1:
                for kc in range(n_dc):
                    src = rgb_rows if kc < DD // 128 else flow_rows
                    dcol = (kc % (DD // 128)) * 128
                    slab = tmp.tile([128, n_rt, 128], fp32, tag="slab",
                                    name="slab")
                    dma(
                        out=slab[:, :, :],
                        in_=src[:, dcol:dcol + 128]
                        .rearrange("(rt p) d -> p rt d", p=128))
                    xc_sb = tmp.tile([128, rp], fp32, tag="xc_sb",
                                     name="xc_sb")
                    for rt in range(n_rt):
                        ps = tps1.tile([128, 128], fp32, tag="tpsa",
                                       name="tpsa")
                        nc.tensor.transpose(ps[:, :], slab[:, rt, :],
                                            ident[:, :])
                        nc.vector.tensor_copy(
                            out=xc_sb[:, rt * 128:(rt + 1) * 128],
                            in_=ps[:, :])
                    dma(
                        out=xcat_dram[kc * 128:(kc + 1) * 128, :],
                        in_=xc_sb[:, :])

            # ---- layer1 + LN (x stays in SBUF), per encoder ----
            x_sb = {e: xsb.tile([128, n_rt, E], fp32, tag=f"x_sb_{e}",
                                name=f"x_sb_{e}") for e in ENCS}
            l1scope = ExitStack()
            l1ps = l1scope.enter_context(
                tc.tile_pool(name="l1ps", bufs=1, space="PSUM"))
            for e in ENCS:
                psl = [[l1ps.tile([128, 512], fp32, tag=f"l1_{rt}_{n}",
                                  name=f"l1_{rt}_{n}")
                        for n in range(2)] for rt in range(n_rt)]
                for kc in range(n_dc):
                    xck = tmp.tile([128, rp], fp32, tag="xck", name="xck")
                    dma(
                        out=xck[:, :].bitcast(f32r),
                        in_=xcat_dram[kc * 128:(kc + 1) * 128,
                                      :].bitcast(f32r))
                    if e == "q":
                        xmm = tmp.tile([128, rp], fp32, tag="xmm",
                                       name="xmm")
                        nc.vector.tensor_mul(xmm[:, :].bitcast(f32r),
                                             xck[:, :], mask_b[:, :])
                        src_chunk = xmm
                    else:
                        src_chunk = xck
                    w1row = tmp.tile([128, E], fp32, tag="w1row",
                                     name="w1row")
                    dma(
                        out=w1row[:, :].bitcast(f32r),
                        in_=w1["q"][kc * 128:(kc + 1) * 128,
                                    :].bitcast(f32r))
                    if e == "k":
                        w1k = tmp.tile([128, E], fp32, tag="w1k", name="w1k")
                        dma(
                            out=w1k[:, :],
                            in_=w1["k"][kc * 128:(kc + 1) * 128, :])
                        w1c = tmp.tile([128, E], fp32, tag="w1c", name="w1c")
                        nc.vector.tensor_scalar_mul(w1c[:, :].bitcast(f32r),
                                                    w1k[:, :], MOM)
                        nc.vector.scalar_tensor_tensor(
                            out=w1c[:, :].bitcast(f32r), in0=w1row[:, :],
                            scalar=1.0 - MOM,
                            in1=w1c[:, :], op0=OP.mult, op1=OP.add)
                        wrow = w1c
                    else:
                        wrow = w1row
                    for rt in range(n_rt):
                        for n in range(2):
                            nc.tensor.matmul(
                                psl[rt][n][:, :],
                                r(src_chunk[:, rt * 128:(rt + 1) * 128]),
                                r(wrow[:, n * 512:(n + 1) * 512]),
                                start=(kc == 0), stop=False)
                for rt in range(n_rt):
                    for n in range(2):
                        nc.tensor.matmul(
                            psl[rt][n][:, :], r(ones_row[:, 0:128]),
                            r(b1row[e][:, n * 512:(n + 1) * 512]),
                            start=False, stop=True)
                # LN + affine into x_sb (relu folded into transpose copy)
                for rt in range(n_rt):
                    stats = lnp.tile([128, 2, 6], fp32, tag="stats",
                                     name="stats")
                    nc.vector.bn_stats(out=stats[:, 0, :],
                                       in_=psl[rt][0][:, :])
                    nc.vector.bn_stats(out=stats[:, 1, :],
                                       in_=psl[rt][1][:, :])
                    mv = lnp.tile([128, 2], fp32, tag="mv", name="mv")
                    nc.vector.bn_aggr(out=mv[:, :], in_=stats[:, :, :])
                    rstd = lnp.tile([128, 1], fp32, tag="rstd", name="rstd")
                    nc.scalar.activation(out=rstd[:, :], in_=mv[:, 1:2],
                                         func=AF.Sqrt, bias=epst[:, :],
                                         scale=1.0)
                    nc.vector.reciprocal(out=rstd[:, :], in_=rstd[:, :])
                    nmr = lnp.tile([128, 1], fp32, tag="nmr", name="nmr")
                    nc.vector.scalar_tensor_tensor(
                        out=nmr[:, :], in0=mv[:, 0:1], scalar=-1.0,
                        in1=rstd[:, :], op0=OP.mult, op1=OP.mult)
                    t1 = a2.tile([128, E], fp32, tag="t1", name="t1")
                    for n in range(2):
                        nc.scalar.activation(
                            out=t1[:, n * 512:(n + 1) * 512],
                            in_=psl[rt][n][:, :], func=AF.Identity,
                            bias=nmr[:, :], scale=rstd[:, :])
                    nc.vector.tensor_mul(t1[:, :], t1[:, :], gb[e][:, :])
                    nc.vector.tensor_add(x_sb[e][:, rt, :], t1[:, :],
                                         beb[e][:, :])
            l1scope.close()

            # ---- x^T transposes (relu folded) + AllGather, per encoder ----
            tps2 = actx.enter_context(tc.tile_pool(name="tps2", bufs=4,
                                                   space="PSUM"))
            for e in ENCS:
                agi = agd.tile([E, rp], fp32, tag="agx_in", name="agx_in")
                for rt in range(n_rt):
                    xtp = a2.tile([128, n_ec, 128], fp32, tag="xtp",
                                  name="xtp")
                    for ec in range(n_ec):
                        ps = tps2.tile([128, 128], fp32, tag="tpsa2",
                                       name="tpsa2")
                        nc.tensor.transpose(
                            ps[:, :],
                            x_sb[e][:, rt, ec * 128:(ec + 1) * 128],
                            ident[:, :])
                        nc.vector.tensor_scalar_max(xtp[:, ec, :],
                                                    ps[:, :], 0.0)
                    dma(
                        out=agi[:, rt * 128:(rt + 1) * 128]
                        .rearrange("(ec p) c -> p ec c", p=128),
                        in_=xtp[:, :, :])
                nc.gpsimd.collective_compute(
                    "AllGather", OP.bypass, replica_groups=RG,
                    ins=[agi[:, :].opt()], outs=[xall[e][:, :, :].opt()])


        # ================= whh^T / wih^T prep + xi GEMMs =================
        # (whh prep first: fills the x-AllGather latency with useful work)
        wscope = ctx.enter_context(tc.tile_pool(name="wscope", bufs=1))
        whhT = {e: wscope.tile([128, n_kc, G3], fp32, tag=f"whhT_{e}",
                               name=f"whhT_{e}") for e in ENCS}
        with ExitStack() as pctx:
            wtmp = pctx.enter_context(tc.tile_pool(name="wtmp", bufs=1))
            ptps = pctx.enter_context(tc.tile_pool(name="ptps", bufs=4,
                                                   space="PSUM"))
            wprep = pctx.enter_context(tc.tile_pool(name="wprep", bufs=1))
            xl = pctx.enter_context(tc.tile_pool(name="xl", bufs=4))
            xa2 = pctx.enter_context(tc.tile_pool(name="xa2", bufs=2))
            xip = pctx.enter_context(tc.tile_pool(name="xip", bufs=2,
                                                  space="PSUM"))

            def prep_transposed(dst, srcs, g, rt2, nchunk, is_k):
                ncols = srcs["q"][g].shape[1]
                wt = wtmp.tile([128, ncols], fp32, tag="wr_a", name="wr_a")
                dma(
                    out=wt[:, :],
                    in_=srcs["q"][g][rt2 * 128:(rt2 + 1) * 128, :])
                if is_k:
                    wtk = wtmp.tile([128, ncols], fp32, tag="wr_b",
                                    name="wr_b")
                    dma(
                        out=wtk[:, :],
                        in_=srcs["k"][g][rt2 * 128:(rt2 + 1) * 128, :])
                    wtc = wtmp.tile([128, ncols], fp32, tag="wr_c",
                                    name="wr_c")
                    nc.vector.tensor_scalar_mul(wtc[:, :], wtk[:, :], MOM)
                    nc.vector.scalar_tensor_tensor(
                        out=wtc[:, :], in0=wt[:, :], scalar=1.0 - MOM,
                        in1=wtc[:, :], op0=OP.mult, op1=OP.add)
                    wt = wtc
                for cc2 in range(nchunk):
                    ps = ptps.tile([128, 128], fp32, tag="ptpsa",
                                   name="ptpsa")
                    nc.tensor.transpose(
                        ps[:, :], wt[:, cc2 * 128:(cc2 + 1) * 128],
                        ident[:, :])
                    nc.vector.tensor_copy(
                        out=dst[:, cc2,
                                g * HS + rt2 * 128:
                                g * HS + (rt2 + 1) * 128].bitcast(f32r),
                        in_=ps[:, :])

            for e in ENCS:
                for g in range(3):
                    for rt2 in range(HS // 128):
                        prep_transposed(whhT[e], whh, g, rt2, n_kc,
                                        e == "k")

            for e in ENCS:
                wihT = wprep.tile([128, n_ec, G3], fp32, tag="wihT",
                                  name="wihT")
                for g in range(3):
                    for rt2 in range(HS // 128):
                        prep_transposed(wihT, wih, g, rt2, n_ec, e == "k")
                for m in range(n_m):
                    jb, sub = divmod(m, n_rt)
                    psA = xip.tile([128, 512], fp32, tag="xipA", name="xipA")
                    psB = xip.tile([128, 256], fp32, tag="xipB", name="xipB")
                    for kc in range(n_ec):
                        lt = xl.tile([128, 128], fp32, tag="lt", name="lt")
                        dma(
                            out=lt[:, :].bitcast(f32r),
                            in_=xall[e][jb, kc * 128:(kc + 1) * 128,
                                        sub * 128:(sub + 1) * 128]
                            .bitcast(f32r))
                        nc.tensor.matmul(psA[:, :], r(lt[:, :]),
                                         r(wihT[:, kc, 0:512]),
                                         start=(kc == 0), stop=False)
                        nc.tensor.matmul(psB[:, :], r(lt[:, :]),
                                         r(wihT[:, kc, 512:G3]),
                                         start=(kc == 0), stop=False)
                    nc.tensor.matmul(psA[:, :], r(ones_row[:, 0:128]),
                                     r(bfrow[e][:, 0:512]),
                                     start=False, stop=True)
                    nc.tensor.matmul(psB[:, :], r(ones_row[:, 0:128]),
                                     r(bfrow[e][:, 512:G3]),
                                     start=False, stop=True)
                    xi_sb = xa2.tile([128, G3], fp32, tag="xi_sb",
                                     name="xi_sb")
                    nc.vector.tensor_copy(out=xi_sb[:, 0:512], in_=psA[:, :])
                    nc.vector.tensor_copy(out=xi_sb[:, 512:G3],
                                          in_=psB[:, :])
                    dma(
                        out=xi_dram[e][m * 128:(m + 1) * 128, :],
                        in_=xi_sb[:, :])

        # ================= PHASE B: recurrence =================
        with ExitStack() as bctx:
            bxi = bctx.enter_context(tc.tile_pool(name="bxi", bufs=2))
            bgate = bctx.enter_context(tc.tile_pool(name="bgate", bufs=2))
            bps = bctx.enter_context(tc.tile_pool(name="bps", bufs=1,
                                                  space="PSUM"))
            btps = bctx.enter_context(tc.tile_pool(name="btps", bufs=2,
                                                   space="PSUM"))
            hT = {e: None for e in ENCS}
            h_prev = {e: None for e in ENCS}
            xi_r = {e: xi_dram[e][:, :].rearrange("(b t) g -> t b g",
                                                  t=t_steps)
                    for e in ENCS}
            for t in range(t_steps):
                # combined AG input: [q slice (256); k slice (256)] x 32
                agi = agd.tile([2 * HS, 32], fp32, tag="agh_in",
                               name="agh_in")
                rzs = {}
                xis = {}
                # 1) r/z matmuls + sigmoid for both encoders first
                for e in ENCS:
                    xi_t = bxi.tile([32, G3], fp32, tag=f"xi_t_{e}",
                                    name=f"xi_t_{e}")
                    dma(out=xi_t[:, :], in_=xi_r[e][t])
                    xis[e] = xi_t
                    trz = bgate.tile([32, 512], fp32, tag=f"trz_{e}",
                                     name=f"trz_{e}")
                    if t > 0:
                        ps_rz = bps.tile([32, 512], fp32, tag=f"psrz_{e}",
                                         name=f"psrz_{e}")
                        for kc in range(n_kc):
                            nc.tensor.matmul(
                                ps_rz[:, :], r(hT[e][:, kc, :]),
                                r(whhT[e][:, kc, 0:512]),
                                start=(kc == 0), stop=(kc == n_kc - 1))
                        nc.vector.tensor_add(trz[:, :], ps_rz[:, :],
                                             xi_t[:, 0:512])
                    else:
                        nc.vector.tensor_copy(out=trz[:, :],
                                              in_=xi_t[:, 0:512])
                    rz = bgate.tile([32, 512], fp32, tag=f"rz_{e}",
                                    name=f"rz_{e}")
                    nc.scalar.activation(out=rz[:, :], in_=trz[:, :],
                                         func=AF.Sigmoid)
                    rzs[e] = rz
                # 2) n matmuls + gate tail + transpose, per encoder
                for ei, e in enumerate(ENCS):
                    xi_t = xis[e]
                    rz = rzs[e]
                    tn = bgate.tile([32, HS], fp32, tag=f"tn_{e}",
                                    name=f"tn_{e}")
                    if t > 0:
                        ps_n = bps.tile([32, 256], fp32, tag=f"psn_{e}",
                                        name=f"psn_{e}")
                        for kc in range(n_kc):
                            nc.tensor.matmul(
                                ps_n[:, :], r(hT[e][:, kc, :]),
                                r(whhT[e][:, kc, 512:G3]),
                                start=(kc == 0), stop=(kc == n_kc - 1))
                        nc.vector.tensor_add(tn[:, :], ps_n[:, :],
                                             bhhnb[e][:, :])
                    else:
                        nc.vector.tensor_copy(out=tn[:, :],
                                              in_=bhhnb[e][:, :])
                    nc.vector.tensor_mul(tn[:, :], rz[:, 0:256], tn[:, :])
                    nc.vector.tensor_add(tn[:, :], tn[:, :], xi_t[:, 512:G3])
                    nn_t = bgate.tile([32, HS], fp32, tag=f"nn_{e}",
                                      name=f"nn_{e}")
                    nc.scalar.activation(out=nn_t[:, :], in_=tn[:, :],
                                         func=AF.Tanh)
                    d_t = bgate.tile([32, HS], fp32, tag=f"d_{e}",
                                     name=f"d_{e}")
                    if t > 0:
                        nc.vector.tensor_sub(d_t[:, :], h_prev[e][:, :],
                                             nn_t[:, :])
                    else:
                        nc.vector.tensor_scalar_mul(d_t[:, :], nn_t[:, :],
                                                    -1.0)
                    nc.vector.tensor_mul(d_t[:, :], rz[:, 256:512],
                                         d_t[:, :])
                    h_new = bh.tile([32, HS], fp32, tag=f"hnew_{e}",
                                    name=f"hnew_{e}")
                    nc.vector.tensor_add(h_new[:, :], nn_t[:, :], d_t[:, :])
                    h_prev[e] = h_new
                    hts = bgate.tile([128, HS // 128, 32], fp32,
                                     tag=f"hts_{e}", name=f"hts_{e}")
                    for hf in range(HS // 128):
                        pst = btps.tile([128, 32], fp32, tag="pst",
                                        name="pst")
                        nc.tensor.transpose(
                            pst[:, :], h_new[:, hf * 128:(hf + 1) * 128],
                            ident[0:32, 0:32])
                        nc.vector.tensor_copy(out=hts[:, hf, :],
                                              in_=pst[:, :])
                    dma(
                        out=agi[ei * HS:(ei + 1) * HS, :]
                        .rearrange("(hf p) c -> p hf c", p=128),
                        in_=hts[:, :, :])
                # 3) one AllGather for both encoders
                ago = agd.tile([NC_N * 2 * HS, 32], fp32, tag="agh_out",
                               name="agh_out", addr_space="Shared")
                nc.gpsimd.collective_compute(
                    "AllGather", OP.bypass, replica_groups=RG,
                    ins=[agi[:, :].opt()], outs=[ago[:, :].opt()])
                for ei, e in enumerate(ENCS):
                    hT_new = bh.tile([128, n_kc, 32], fp32, tag=f"hT_{e}",
                                     name=f"hT_{e}")
                    for hf in range(HS // 128):
                        src_ap = bass.AP(
                            tensor=ago.tensor,
                            offset=(ago.offset + ei * HS * 32
                                    + hf * 128 * 32),
                            ap=[[32, 128], [2 * HS * 32, NC_N], [1, 32]])
                        out_ap = bass.AP(
                            tensor=hT_new.tensor,
                            offset=hT_new.offset + hf * 32,
                            ap=[[n_kc * 32, 128], [2 * 32, NC_N], [1, 32]])
                        dma(
                            out=out_ap.bitcast(f32r),
                            in_=src_ap.bitcast(f32r))
                    hT[e] = hT_new

        # ================= PHASE C: head + queues =================
        with ExitStack() as cctx:
            cp = cctx.enter_context(tc.tile_pool(name="cp", bufs=2))
            cps = cctx.enter_context(tc.tile_pool(name="cps", bufs=2,
                                                  space="PSUM"))
            qt = cctx.enter_context(tc.tile_pool(name="qt", bufs=3))
            qconst = cctx.enter_context(tc.tile_pool(name="qconst", bufs=1))

            s_sb = qconst.tile([B, C, KS], fp32, tag="s_sb", name="s_sb")
            dma(
                out=s_sb[:, :, :].bitcast(f32r),
                in_=s_in.ap().rearrange("c b k -> b c k").bitcast(f32r))
            mnot_b = qconst.tile([128, C, KS], fp32, tag="mnot_b",
                                 name="mnot_b")
            dma(out=mnot_b[:, :, :], in_=bcast(mnot_in.ap()))
            qsb = qconst.tile([128, C, KS], fp32, tag="qsb", name="qsb")
            dma(out=qsb[:, :, :],
                              in_=queues_in.ap().rearrange("c p k -> p c k"))

            cls_sb = {}
            for e in ENCS:
                featT = cp.tile([128, n_kc, 32], fp32, tag=f"featT_{e}",
                                name=f"featT_{e}")
                nc.scalar.activation(out=featT[:, :, :].bitcast(f32r),
                                     in_=hT[e][:, :, :], func=AF.Relu)
                ps_cls = cps.tile([32, CD], fp32, tag=f"pscls_{e}",
                                  name=f"pscls_{e}")
                for kc in range(n_kc):
                    nc.tensor.matmul(ps_cls[:, :], r(featT[:, kc, :]),
                                     r(wq_sb[:, kc, :]),
                                     start=(kc == 0), stop=False)
                nc.tensor.matmul(ps_cls[:, :], r(ones_row[:, 0:32]),
                                 r(bq_row[:, :]), start=False, stop=True)
                sq = cp.tile([32, CD], fp32, tag=f"sq_{e}", name=f"sq_{e}")
                ssum = cp.tile([32, 1], fp32, tag=f"ssum_{e}",
                               name=f"ssum_{e}")
                nc.scalar.activation(out=sq[:, :], in_=ps_cls[:, :],
                                     func=AF.Square, accum_out=ssum[:, :])
                rn = cp.tile([32, 1], fp32, tag=f"rn_{e}", name=f"rn_{e}")
                nc.scalar.activation(out=rn[:, :], in_=ssum[:, :],
                                     func=AF.Sqrt)
                nc.vector.reciprocal(out=rn[:, :], in_=rn[:, :])
                cls = cp.tile([32, CD], fp32, tag=f"cls_{e}",
                              name=f"cls_{e}")
                nc.scalar.activation(out=cls[:, :], in_=ps_cls[:, :],
                                     func=AF.Copy, scale=rn[:, :])
                cls_sb[e] = cls
                out_t = q_cls_out if e == "q" else k_cls_out
                dma(out=out_t[:, :], in_=cls[:, :])

            clsr = cp.tile([32, CD], fp32, tag="clsr", name="clsr")
            nc.vector.tensor_copy(out=clsr[:, :].bitcast(f32r),
                                  in_=cls_sb["k"][:, :])
            for c in range(C):
                psq = cps.tile([128, KS], fp32, tag="psq", name="psq")
                nc.tensor.matmul(psq[:, :], r(clsr[:, :]),
                                 r(s_sb[:, c, :]), start=True, stop=True)
                qn = qt.tile([128, KS], fp32, tag="qn", name="qn")
                nc.vector.tensor_mul(qn[:, :], qsb[:, c, :],
                                     mnot_b[:, c, :])
                nc.vector.tensor_add(qn[:, :], qn[:, :], psq[:, :])
                dma(out=newq_out[c, :, :], in_=qn[:, :])

    nc.compile()
    return nc


def _prep_inputs(inputs, t_steps=T, rp=RP):
    """Build the 8 per-core input maps from the full input dict."""
    f = {k: np.asarray(v) for k, v in inputs.items()}
    rgb = f["rgb"].astype(np.float32, copy=False)
    flow = f["flow"].astype(np.float32, copy=False)
    rand_mask = f["rand_mask"].astype(np.float32, copy=False)
    targets = f["targets"].astype(np.float32, copy=False)
    ptrs = f["ptrs"].astype(np.int64)
    queues = f["queues"].astype(np.float32, copy=False)

    b_ = rgb.shape[0]
    mask = (rand_mask[:, :, 0] > MASK_RATIO).astype(np.float32)
    mask[:, -1] = 1.0

    def mu(kp, qp):
        return (MOM * f[kp].astype(np.float64)
                + (1.0 - MOM) * f[qp].astype(np.float64)).astype(np.float32)

    b1_k = mu("b1_k", "b1_q")
    g_k = mu("g_k", "g_q")
    be_k = mu("be_k", "be_q")
    bih_k = mu("bih_k", "bih_q")
    bhh_k = mu("bhh_k", "bhh_q")
    bih_q = f["bih_q"].astype(np.float32, copy=False)
    bhh_q = f["bhh_q"].astype(np.float32, copy=False)

    sel = targets > 0.5
    pos = np.cumsum(sel, axis=0) - 1
    slot = (ptrs[None, :].astype(np.int64) + pos) % KQ
    cnt = sel.sum(0).astype(np.int64)
    new_ptrs = ((ptrs + cnt) % KQ).astype(np.int32)

    S = np.zeros((NC_N, C, b_, KS), np.float32)
    for bb in range(b_):
        for cc in range(C):
            if sel[bb, cc]:
                s = int(slot[bb, cc])
                S[s // KS, cc, bb, s % KS] = 1.0
    Mnot = 1.0 - S.sum(axis=2)  # [NC, C, KS]

    bpc = b_ // NC_N
    in_maps = []
    for j in range(NC_N):
        m = {
            "rgb_rows": np.ascontiguousarray(
                rgb[j * bpc:(j + 1) * bpc].reshape(rp, -1)),
            "flow_rows": np.ascontiguousarray(
                flow[j * bpc:(j + 1) * bpc].reshape(rp, -1)),
            "mask_rows": np.ascontiguousarray(
                mask[j * bpc:(j + 1) * bpc].reshape(rp)),
            "w1_q": f["w1_q"], "w1_k": f["w1_k"],
            "b1_q": f["b1_q"], "b1_k": b1_k,
            "g_q": f["g_q"], "g_k": g_k,
            "be_q": f["be_q"], "be_k": be_k,
            "wq": f["wq"], "bq": f["bq"],
            "queues_j": np.ascontiguousarray(
                queues[:, :, j * KS:(j + 1) * KS]),
            "S_j": np.ascontiguousarray(S[j]),
            "Mnot_j": np.ascontiguousarray(Mnot[j]),
        }
        hh = H
        for e in ("q", "k"):
            wihf = f[f"wih_{e}"]
            whhf = f[f"whh_{e}"]
            for g in range(3):
                sl = slice(g * hh + j * HS, g * hh + (j + 1) * HS)
                m[f"wih_{e}_{g}"] = np.ascontiguousarray(wihf[sl])
                m[f"whh_{e}_{g}"] = np.ascontiguousarray(whhf[sl])
        for e, bihv, bhhv in (("q", bih_q, bhh_q), ("k", bih_k, bhh_k)):
            bf = np.empty(G3, np.float32)
            for g in range(3):
                sl = slice(g * hh + j * HS, g * hh + (j + 1) * HS)
                bf[g * HS:(g + 1) * HS] = bihv[sl]
                if g < 2:  # bhh for r,z folded; n-gate bhh applied in-step
                    bf[g * HS:(g + 1) * HS] += bhhv[sl]
            m[f"biasfold_{e}"] = bf
            m[f"bhhn_{e}"] = np.ascontiguousarray(
                bhhv[2 * hh + j * HS: 2 * hh + (j + 1) * HS])
        in_maps.append(m)
    return in_maps, new_ptrs


def _assemble(results, new_ptrs):
    q_cls = np.asarray(results[0]["q_cls"])
    k_cls = np.asarray(results[0]["k_cls"])
    new_queues = np.concatenate(
        [np.asarray(results[j]["newq"]) for j in range(NC_N)], axis=2)
    return q_cls, k_cls, new_queues, new_ptrs


def kernel(**inputs):
    from concourse import bass_utils
    if "nc" not in _CACHE:
        _CACHE["nc"] = _build()
    nc = _CACHE["nc"]
    in_maps, new_ptrs = _prep_inputs(inputs)
    res = bass_utils.run_bass_kernel_spmd(nc, in_maps,
                                          core_ids=list(range(NC_N)))
    return _assemble(res.results, new_ptrs)
